# revision 1
# baseline (speedup 1.0000x reference)
"""Multi-head causal self-attention on 8 TRN2 NeuronCores.

Problem: B=2, T=4096, D=512, H=8 heads (hd=64), fp32 in/out.

Sharding: core c in 0..7 handles batch b = c//4 and head pair g = c%4
(heads 2g, 2g+1 -> D-slice [128g, 128g+128)). Each core computes
    partial_out = concat_h( softmax(causal(Q_h K_h^T / 8)) V_h ) @ W_O[slice]
for its two heads; the host sums the 4 partials per batch and adds b_O.

On-core dataflow (all matmul operands bf16, f32 PSUM accumulation):
  - X^T (host-pretransposed) streams in as 4 [128,4096] chunks.
  - Q^T,K^T [128(d-pair),4096] = W_chunk^T @ X^T, bias added during the
    PSUM->SBUF bf16 copy (per-partition scalar add on DVE).
  - V [4096,64+1] per head in natural layout (ones column appended ->
    the attention row-sum L falls out of the PV matmul for free).
  - Scores are computed transposed, S^T[k-block, q] (contraction over the
    64-dim head axis; the two heads run in disjoint PE row groups), causally
    streamed: for key block kb only q >= 128*kb is computed. exp() runs on
    ScalarE straight out of PSUM with the 1/8 scale folded in; the diagonal
    128x128 subtile is masked by accumulating -1e9 upper-triangle via an
    identity matmul before the exp.
  - Z^T_aug[65, q] accumulates P^T-block x V_aug over key blocks in PSUM;
    row 64 is L. The S->exp->PV chain is software-pipelined: scores run one
    group ahead of the PV matmuls, and PSUM group tiles are triple-buffered,
    which keeps TensorE dense enough that the HAM clock-gate mostly stays at
    2.4 GHz.
  - Normalisation (off the critical path): Z_aug is evacuated to SBUF at
    once (freeing the PSUM bank for the next slice), 1/L runs on a [128,4]
    partition-spread layout (DVE iterative divide costs freedim x 8 cycles),
    and the broadcast across partitions goes through a DRAM bounce (step-0
    partition APs are only legal on DRAM sources).
  - O-projection: lhsT = stacked [Z_A; Z_B] [128, t-tile] (head B shifted to
    partitions 64..127 via SBUF->SBUF DMA), rhs = W_O pair [128,512].
    Each slice's O-projection is emitted AFTER the next slice's attention
    groups so its normalisation chain never stalls the PE stream.
"""

import numpy as np

import concourse.bass as bass
import concourse.mybir as mybir
from concourse.tile import TileContext
from concourse.bass_utils import run_bass_kernel_spmd

try:
    import ml_dtypes

    _BF16 = ml_dtypes.bfloat16
except ImportError:  # pragma: no cover
    _BF16 = None

F32 = mybir.dt.float32
BF16 = mybir.dt.bfloat16

B, T, D, H = 2, 4096, 512, 8
HD = D // H  # 64
SW = 512  # q-slice width
NS = T // SW  # 8 q-slices
NKC = D // 128  # 4 contraction chunks for the projections
NTT = T // 128  # 32 t-tiles / key blocks
GK = 2  # key blocks grouped per exp() call (2 PSUM banks)
NEG = -1.0e9


def _split_waits(nc, max_waits=1):
    """The staged walrus rejects >1 semaphore wait per instruction; hoist
    extras onto same-engine NoOps inserted right before the instruction."""
    counter = 0
    for f in nc.m.functions:
        for blk in f.blocks:
            insts = blk.instructions
            out, changed = [], False
            for ins in insts:
                si = getattr(ins, "sync_info", None)
                waits = list(si.on_wait) if si is not None and si.on_wait else []
                if len(waits) > max_waits:
                    changed = True
                    for w in waits[:-max_waits]:
                        counter += 1
                        nop = mybir.InstNoOp(
                            name=f"I-wsplit-{counter}",
                            engine=ins.engine,
                            ins=[],
                            outs=[],
                        )
                        nop.sync_info = mybir.SyncInfo(on_wait=[w], on_update=[])
                        out.append(nop)
                    ins.sync_info = mybir.SyncInfo(
                        on_wait=waits[-max_waits:], on_update=list(si.on_update)
                    )
                out.append(ins)
            if changed:
                blk.instructions = out
    return counter


def build_nc():
    nc = bass.Bass("TRN2")

    xt = nc.dram_tensor("xt", [D, T], BF16, kind="ExternalInput")
    wq = nc.dram_tensor("wq", [D, 128], BF16, kind="ExternalInput")
    wk = nc.dram_tensor("wk", [D, 128], BF16, kind="ExternalInput")
    wv = nc.dram_tensor("wv", [D, 128], BF16, kind="ExternalInput")
    wo = nc.dram_tensor("wo", [128, D], BF16, kind="ExternalInput")
    bq = nc.dram_tensor("bq", [128, 1], F32, kind="ExternalInput")
    bk = nc.dram_tensor("bk", [128, 1], F32, kind="ExternalInput")
    bv = nc.dram_tensor("bv", [1, 128], BF16, kind="ExternalInput")
    out = nc.dram_tensor("out", [T, D], F32, kind="ExternalOutput")

    ident_np = np.eye(128, dtype=np.float32)
    # maskneg[k, q'] = 0 where q' >= k else NEG  (S^T diagonal subtile mask)
    mask_np = np.where(
        np.arange(128)[None, :] >= np.arange(128)[:, None], 0.0, NEG
    ).astype(np.float32)
    ident_dram = nc.inline_tensor(ident_np.astype(_BF16), name="identc")
    mask_dram = nc.inline_tensor(mask_np.astype(_BF16), name="maskc")

    with TileContext(nc) as tc:
        with (
            tc.tile_pool(name="singles", bufs=1) as singles,
            tc.tile_pool(name="ps", bufs=3, space="PSUM") as ps,
            tc.tile_pool(name="zps", bufs=1, space="PSUM") as zps,
            tc.tile_pool(name="pt", bufs=6) as ptp,
            tc.tile_pool(name="sl", bufs=3) as slp,
            tc.tile_pool(name="outp", bufs=4) as outp,
            tc.tile_pool(name="drp", bufs=2, space="DRAM") as drp,
        ):
            # ---- static SBUF ----
            xt_sb = [
                [
                    singles.tile(
                        [128, SW], BF16, tag=f"xt{c}_{s}", name=f"xt_sb{c}_{s}"
                    )
                    for s in range(NS)
                ]
                for c in range(NKC)
            ]
            for s in range(NS):
                for c in range(NKC):
                    nc.sync.dma_start(
                        out=xt_sb[c][s][:, :],
                        in_=xt[c * 128 : (c + 1) * 128, s * SW : (s + 1) * SW],
                    )

            wq_sb = singles.tile([128, NKC, 128], BF16, tag="wq")
            wk_sb = singles.tile([128, NKC, 128], BF16, tag="wk")
            wv_sb = singles.tile([128, NKC, 128], BF16, tag="wv")
            for c in range(NKC):
                nc.sync.dma_start(out=wq_sb[:, c, :], in_=wq[c * 128 : (c + 1) * 128, :])
                nc.sync.dma_start(out=wk_sb[:, c, :], in_=wk[c * 128 : (c + 1) * 128, :])
                nc.sync.dma_start(out=wv_sb[:, c, :], in_=wv[c * 128 : (c + 1) * 128, :])
            wo_sb = singles.tile([128, D], BF16, tag="wo")
            nc.sync.dma_start(out=wo_sb[:, :], in_=wo[:, :])

            bq_sb = singles.tile([128, 1], F32, tag="bq")
            bk_sb = singles.tile([128, 1], F32, tag="bk")
            bv_sb = singles.tile([1, 512], BF16, tag="bv")
            nc.sync.dma_start(out=bq_sb[:, :], in_=bq[:, :])
            nc.sync.dma_start(out=bk_sb[:, :], in_=bk[:, :])
            for j in range(4):
                nc.sync.dma_start(out=bv_sb[:, j * 128 : (j + 1) * 128], in_=bv[:, :])

            ident_sb = singles.tile([128, 128], BF16, tag="ident")
            mask_sb = singles.tile([128, 128], BF16, tag="mask")
            nc.sync.dma_start(out=ident_sb[:, :], in_=ident_dram[:, :])
            nc.sync.dma_start(out=mask_sb[:, :], in_=mask_dram[:, :])
            ones_sb = singles.tile([1, 128], BF16, tag="ones")
            nc.vector.memset(ones_sb[:, :], 1.0)

            qt_sb = [
                singles.tile([128, SW], BF16, tag=f"qt{s}", name=f"qt_sb{s}")
                for s in range(NS)
            ]
            kt_sb = [
                singles.tile([128, SW], BF16, tag=f"kt{s}", name=f"kt_sb{s}")
                for s in range(NS)
            ]
            # V_aug per head per key block: [128(t), 65]; col 64 = ones
            va_sb = [
                singles.tile([128, HD + 1], BF16, tag=f"va{t}", name=f"va_sb{t}")
                for t in range(NTT)
            ]
            vb_sb = [
                singles.tile([128, HD + 1], BF16, tag=f"vb{t}", name=f"vb_sb{t}")
                for t in range(NTT)
            ]
            ones_row = singles.tile([1, SW], F32, tag="onesrow")
            nc.vector.memset(ones_row[:, :], 1.0)

            # ---- QKV projections (emitted per q-slice, interleaved with
            # attention so ScalarE starts exp-ing early) ----
            def emit_qkv(s):
                cols = slice(s * SW, (s + 1) * SW)
                ps_q = ps.tile([128, SW], F32, tag="sg", name="ps_q")
                for c in range(NKC):
                    nc.tensor.matmul(
                        ps_q[:, :],
                        lhsT=wq_sb[:, c, :],
                        rhs=xt_sb[c][s][:, :],
                        start=(c == 0),
                        stop=(c == NKC - 1),
                        skip_group_check=True,
                    )
                nc.vector.tensor_scalar_add(qt_sb[s][:, :], ps_q[:, :], bq_sb[:, :])
                ps_k = ps.tile([128, SW], F32, tag="sg", name="ps_k")
                for c in range(NKC):
                    nc.tensor.matmul(
                        ps_k[:, :],
                        lhsT=wk_sb[:, c, :],
                        rhs=xt_sb[c][s][:, :],
                        start=(c == 0),
                        stop=(c == NKC - 1),
                        skip_group_check=True,
                    )
                nc.vector.tensor_scalar_add(kt_sb[s][:, :], ps_k[:, :], bk_sb[:, :])
                for t in range(4 * s, 4 * s + 4):
                    tloc = slice((t % 4) * 128, (t % 4 + 1) * 128)
                    ps_v = ps.tile([128, 128], F32, tag="sg", name="ps_v")
                    for c in range(NKC):
                        nc.tensor.matmul(
                            ps_v[:, :],
                            lhsT=xt_sb[c][s][:, tloc],
                            rhs=wv_sb[:, c, :],
                            start=(c == 0),
                            stop=False,
                            skip_group_check=True,
                        )
                    # + b_V broadcast over rows:  ones[1,128]^T @ bv[1,128]
                    nc.tensor.matmul(
                        ps_v[:, :],
                        lhsT=ones_sb[:, :],
                        rhs=bv_sb[:, 0:128],
                        start=False,
                        stop=True,
                        skip_group_check=True,
                    )
                    nc.vector.tensor_copy(va_sb[t][:, 0:HD], ps_v[:, 0:HD])
                    nc.vector.tensor_copy(vb_sb[t][:, 0:HD], ps_v[:, HD:128])
                    nc.vector.memset(va_sb[t][:, HD : HD + 1], 1.0)
                    nc.vector.memset(vb_sb[t][:, HD : HD + 1], 1.0)

            # ---- attention ----
            vmat = (va_sb, vb_sb)
            hrows = (slice(0, HD), slice(HD, 128))

            def emit_oproj(znpair_t, qs_t):
                for j in range(4):
                    ps_o = ps.tile([128, D], F32, tag="sg", name="ps_o")
                    nc.tensor.matmul(
                        ps_o[:, :],
                        lhsT=znpair_t[:, j * 128 : (j + 1) * 128],
                        rhs=wo_sb[:, :],
                        start=True,
                        stop=True,
                        skip_group_check=True,
                    )
                    o_sb = outp.tile([128, D], F32, tag="ot", name="o_sb")
                    nc.vector.tensor_copy(o_sb[:, :], ps_o[:, :])
                    r0 = qs_t + j * 128
                    nc.sync.dma_start(out=out[r0 : r0 + 128, :], in_=o_sb[:, :])

            pending = []
            for s in range(NS):
                emit_qkv(s)
                qs = s * SW
                nkb = 4 * (s + 1)
                zaug = [
                    zps.tile([HD + 1, SW], F32, tag="za", name="zauga"),
                    zps.tile([HD + 1, SW], F32, tag="zb", name="zaugb"),
                ]
                # pack key blocks tightly into groups; a matmul output may
                # not cross a PSUM bank boundary, so bump to the next bank
                # when a block would straddle one
                groups, cur, cur_cols = [], [], 0
                for kb in range(nkb):
                    qlo = max(qs, kb * 128)
                    n = qs + SW - qlo
                    off = cur_cols
                    if off % SW + n > SW:
                        off = ((off + SW - 1) // SW) * SW
                    if off + n > GK * SW:
                        groups.append(cur)
                        cur, off = [], 0
                    cur.append((kb, off, n, qlo))
                    cur_cols = off + n
                if cur:
                    groups.append(cur)
                def emit_av(av):
                    pt_t, grp_t = av
                    for h in range(2):
                        for kb, off, n, qlo in grp_t:
                            nc.tensor.matmul(
                                zaug[h][0 : HD + 1, qlo - qs : SW],
                                lhsT=vmat[h][kb][:, :],
                                rhs=pt_t[h][:, off : off + n],
                                start=(kb == 0),
                                stop=(kb == nkb - 1),
                                skip_group_check=True,
                            )

                av_queue = []
                for grp in groups:
                    used = grp[-1][1] + grp[-1][2]
                    sg = [None, None]
                    pt = [None, None]
                    for h in range(2):
                        sg[h] = ps.tile([128, GK * SW], F32, tag="sg", name="sg")
                        pt[h] = ptp.tile([128, GK * SW], BF16, tag="pt", name="pt")
                    # scores (both heads interleaved -> disjoint PE row groups)
                    for kb, off, n, qlo in grp:
                        diag = kb * 128 >= qs
                        for h in range(2):
                            nc.tensor.matmul(
                                sg[h][:, off : off + n],
                                lhsT=kt_sb[kb // 4][hrows[h], (kb % 4) * 128 : (kb % 4 + 1) * 128],
                                rhs=qt_sb[s][hrows[h], qlo - qs : qlo - qs + n],
                                start=True,
                                stop=not diag,
                                skip_group_check=True,
                                tile_position=(h * HD, 0),
                            )
                        if diag:
                            for h in range(2):
                                nc.tensor.matmul(
                                    sg[h][:, off : off + 128],
                                    lhsT=ident_sb[:, :],
                                    rhs=mask_sb[:, :],
                                    start=False,
                                    stop=True,
                                    skip_group_check=True,
                                )
                    for h in range(2):
                        nc.scalar.activation(
                            out=pt[h][:, 0:used],
                            in_=sg[h][:, 0:used],
                            func=mybir.ActivationFunctionType.Exp,
                            scale=0.125,
                        )
                    av_queue.append((pt, grp))
                    if len(av_queue) > 1:
                        emit_av(av_queue.pop(0))
                while av_queue:
                    emit_av(av_queue.pop(0))

                # earlier slices' O-projections: their normalisation chains
                # have had >=1 slice of compute to finish -> PE never stalls
                if s >= 2:
                    for p in pending:
                        emit_oproj(*p)
                    pending = []

                last = s == NS - 1
                # evacuate Z^T_aug to SBUF right away (frees the PSUM bank);
                # L row lands at partition 0 so GpSimd ops are partition-aligned.
                # For the final slice latency matters more than PSUM reuse, so
                # work straight off PSUM and use the one-shot reciprocal.
                zsb = [None, None]
                lrow = [None, None]
                if not last:
                    for h in range(2):
                        zsb[h] = slp.tile([HD, SW], F32, tag=f"zsb{h}", name="zsb")
                        nc.vector.tensor_copy(zsb[h][:, :], zaug[h][0:HD, :])
                        lrow[h] = slp.tile([1, SW], F32, tag=f"lr{h}", name="lrow")
                        nc.vector.tensor_copy(lrow[h][:, :], zaug[h][HD : HD + 1, :])

                # normalise z[:, q] / L[q]; the reciprocal runs on a
                # [128, 4] partition-spread layout (DVE iterative divide
                # costs free-dim x 8 cycles, so spread the 512 elements)
                znpair = slp.tile([128, SW], BF16, tag="zn")
                znb = slp.tile([HD, SW], BF16, tag="znb")
                for h in range(2):
                    rd2 = drp.tile([1, SW], F32, tag=f"rd2{h}", name="rd2")
                    if last:
                        rsp1 = slp.tile([1, SW], F32, tag=f"rs1{h}", name="rsp1")
                        nc.vector.reciprocal(rsp1[:, :], zaug[h][HD : HD + 1, :])
                        nc.sync.dma_start(out=rd2[:, :], in_=rsp1[:, :])
                    else:
                        rd = drp.tile([1, SW], F32, tag=f"rd{h}", name="rd")
                        nc.sync.dma_start(out=rd[:, :], in_=lrow[h][:, :])
                        lsp = slp.tile(
                            [128, SW // 128], F32, tag=f"lsp{h}", name="lsp"
                        )
                        nc.sync.dma_start(
                            out=lsp[:, :],
                            in_=rd[0, :].rearrange("(p f) -> p f", p=128),
                        )
                        rsp = slp.tile(
                            [128, SW // 128], F32, tag=f"rsp{h}", name="rsp"
                        )
                        nc.vector.reciprocal(rsp[:, :], lsp[:, :])
                        nc.sync.dma_start(
                            out=rd2[0, :].rearrange("(p f) -> p f", p=128),
                            in_=rsp[:, :],
                        )
                    bc = slp.tile([HD, SW], F32, tag=f"bc{h}")
                    rap = rd2[:, :]
                    bcast_src = bass.AP(
                        tensor=rap.tensor,
                        offset=rap.offset,
                        ap=[[0, HD]] + list(rap.ap[1:]),
                    )
                    nc.sync.dma_start(out=bc[:, :], in_=bcast_src)
                    dst = znpair[0:HD, :] if h == 0 else znb[:, :]
                    src_z = zaug[h][0:HD, :] if last else zsb[h][:, :]
                    nc.vector.tensor_mul(dst, src_z, bc[:, :])
                # move head B rows into partitions 64..127
                nc.gpsimd.dma_start(out=znpair[HD:128, :], in_=znb[:, :])
                pending.append((znpair, qs))

            for p in pending:
                emit_oproj(*p)

    _split_waits(nc)
    return nc


_NC_CACHE = {}


def _get_nc():
    if "nc" not in _NC_CACHE:
        _NC_CACHE["nc"] = build_nc()
    return _NC_CACHE["nc"]


def make_in_maps(combined_embed, W_K, b_K, W_Q, b_Q, W_V, b_V, W_O, b_O):
    f32 = np.float32
    in_maps = []
    for c in range(8):
        b = c // 4
        g = c % 4
        sl = slice(g * 128, (g + 1) * 128)
        xt = np.ascontiguousarray(np.asarray(combined_embed[b], f32).T)
        in_maps.append(
            {
                "xt": xt.astype(_BF16),
                "wq": np.ascontiguousarray(np.asarray(W_Q, f32)[:, sl]).astype(_BF16),
                "wk": np.ascontiguousarray(np.asarray(W_K, f32)[:, sl]).astype(_BF16),
                "wv": np.ascontiguousarray(np.asarray(W_V, f32)[:, sl]).astype(_BF16),
                "wo": np.ascontiguousarray(np.asarray(W_O, f32)[sl, :]).astype(_BF16),
                "bq": np.asarray(b_Q, f32)[sl].reshape(128, 1).copy(),
                "bk": np.asarray(b_K, f32)[sl].reshape(128, 1).copy(),
                "bv": np.asarray(b_V, f32)[sl].reshape(1, 128).astype(_BF16),
            }
        )
    return in_maps


def run_cores(in_maps, **kwargs):
    nc = _get_nc()
    return run_bass_kernel_spmd(nc, in_maps, core_ids=list(range(8)), **kwargs)


def kernel(
    combined_embed, W_K, b_K, W_Q, b_Q, W_V, b_V, W_O, b_O
):  # full inputs -> full output
    in_maps = make_in_maps(
        combined_embed, W_K, b_K, W_Q, b_Q, W_V, b_V, W_O, b_O
    )
    res = run_cores(in_maps)
    out = np.zeros((B, T, D), np.float32)
    for c in range(8):
        out[c // 4] += res.results[c]["out"]
    out += np.asarray(b_O, np.float32)[None, None, :]
    return out



# revision 5
# speedup vs baseline: 1.2379x; 1.2379x over previous
"""Multi-head causal self-attention on 8 TRN2 NeuronCores.

Problem: B=2, T=4096, D=512, H=8 heads (hd=64), fp32 in/out.

Sharding: core c in 0..7 handles batch b = c//4 and head pair g = c%4
(heads 2g, 2g+1 -> D-slice [128g, 128g+128)). Each core computes
    partial_out = concat_h( softmax(causal(Q_h K_h^T / 8)) V_h ) @ W_O[slice]
for its two heads; the host sums the 4 partials per batch and adds b_O.

Pipeline design (v2). ScalarE exp() is the hard floor (~17M causal score
elements per core at 1 elem/cycle/lane @1.2GHz ~= 113us + per-ACTIVATE
overhead); TensorE is a close second (~135us warm) but ran at the cold
1.2GHz HAM clock half the time in v1 because per-slice serialization left
multi-us PE idle gaps every slice.  v2 restructures for dense streams:

  - Score slot = ONE key block x BOTH heads in one [128,1024] PSUM tile
    (head A bank 0, head B bank 1), so each slot needs ONE exp call
    (FD up to 1024 through a [128,2,n] strided AP) -> 144 calls total.
  - Scores run 2 units ahead of exp (3-buf PSUM slot rotation); PV runs
    1 unit behind.  PE order per unit: scores(u+2) first, then PV(u-1),
    so exp's input is never the last thing PE produces.
  - QKV projection (slice s+1), O-projection (slice s-2) and the V
    evacuations are emitted MID-slice as extra rotation slots, filling
    PE/DVE idle and never gating the exp stream at slice boundaries.
  - V bias is a DVE broadcast-add fused into the V PSUM->SBUF evac
    (no more ones-row bias matmuls); zaug evac is one [65,512] copy.
  - exp table set is preloaded with a dummy 1-element call during the
    initial DMA phase.

Normalisation (off the critical path, O-proj deferred 2 slices): L row
travels zsb -> DRAM -> [128,4] partition spread -> DVE reciprocal ->
DRAM -> partition-broadcast [64,512], then one tensor_mul per head; head
B is shifted to partitions 64..127 via GpSimd SBUF-SBUF DMA.
"""

import numpy as np

import concourse.bass as bass
import concourse.mybir as mybir
from concourse.tile import TileContext
from concourse.bass_utils import run_bass_kernel_spmd

try:
    import ml_dtypes

    _BF16 = ml_dtypes.bfloat16
except ImportError:  # pragma: no cover
    _BF16 = None

F32 = mybir.dt.float32
BF16 = mybir.dt.bfloat16

B, T, D, H = 2, 4096, 512, 8
HD = D // H  # 64
SW = 512  # q-slice width
NS = T // SW  # 8 q-slices
NKC = D // 128  # 4 contraction chunks for the projections
NTT = T // 128  # 32 t-tiles / key blocks
NEG = -1.0e9


def _split_waits(nc, max_waits=1):
    """The staged walrus rejects >1 semaphore wait per instruction; hoist
    extras onto same-engine NoOps inserted right before the instruction."""
    counter = 0
    for f in nc.m.functions:
        for blk in f.blocks:
            insts = blk.instructions
            out, changed = [], False
            for ins in insts:
                si = getattr(ins, "sync_info", None)
                waits = list(si.on_wait) if si is not None and si.on_wait else []
                if len(waits) > max_waits:
                    changed = True
                    for w in waits[:-max_waits]:
                        counter += 1
                        nop = mybir.InstNoOp(
                            name=f"I-wsplit-{counter}",
                            engine=ins.engine,
                            ins=[],
                            outs=[],
                        )
                        nop.sync_info = mybir.SyncInfo(on_wait=[w], on_update=[])
                        out.append(nop)
                    ins.sync_info = mybir.SyncInfo(
                        on_wait=waits[-max_waits:], on_update=list(si.on_update)
                    )
                out.append(ins)
            if changed:
                blk.instructions = out
    return counter


def build_nc():
    nc = bass.Bass("TRN2")

    xt = nc.dram_tensor("xt", [D, T], BF16, kind="ExternalInput")
    wq = nc.dram_tensor("wq", [D, 128], BF16, kind="ExternalInput")
    wk = nc.dram_tensor("wk", [D, 128], BF16, kind="ExternalInput")
    wv = nc.dram_tensor("wv", [D, 128], BF16, kind="ExternalInput")
    wo = nc.dram_tensor("wo", [128, D], BF16, kind="ExternalInput")
    bq = nc.dram_tensor("bq", [128, 1], F32, kind="ExternalInput")
    bk = nc.dram_tensor("bk", [128, 1], F32, kind="ExternalInput")
    bv = nc.dram_tensor("bv", [1, 128], F32, kind="ExternalInput")
    out = nc.dram_tensor("out", [T, D], F32, kind="ExternalOutput")

    # maskneg[k, q'] = 0 where q' >= k else NEG  (S^T diagonal subtile mask)
    ident_np = np.eye(128, dtype=np.float32)
    mask_np = np.where(
        np.arange(128)[None, :] >= np.arange(128)[:, None], 0.0, NEG
    ).astype(np.float32)
    ident_dram = nc.inline_tensor(ident_np.astype(_BF16), name="identc")
    mask_dram = nc.inline_tensor(mask_np.astype(_BF16), name="maskc")

    with TileContext(nc) as tc:
        with (
            tc.tile_pool(name="singles", bufs=1) as singles,
            tc.tile_pool(name="ps", bufs=3, space="PSUM") as ps,
            tc.tile_pool(name="zps", bufs=1, space="PSUM") as zps,
            tc.tile_pool(name="pt", bufs=6) as ptp,
            tc.tile_pool(name="sl", bufs=2) as slp,
            tc.tile_pool(name="zn", bufs=3) as znp,
            tc.tile_pool(name="outp", bufs=4) as outp,
            tc.tile_pool(name="drp", bufs=2, space="DRAM") as drp,
        ):
            # ---- static SBUF ----
            wq_sb = singles.tile([128, NKC, 128], BF16, tag="wq")
            wk_sb = singles.tile([128, NKC, 128], BF16, tag="wk")
            wv_sb = singles.tile([128, NKC, 128], BF16, tag="wv")
            for c in range(NKC):
                nc.sync.dma_start(out=wq_sb[:, c, :], in_=wq[c * 128 : (c + 1) * 128, :])
                nc.sync.dma_start(out=wk_sb[:, c, :], in_=wk[c * 128 : (c + 1) * 128, :])
                nc.sync.dma_start(out=wv_sb[:, c, :], in_=wv[c * 128 : (c + 1) * 128, :])
            wo_sb = singles.tile([128, D], BF16, tag="wo")
            nc.sync.dma_start(out=wo_sb[:, :], in_=wo[:, :])

            bq_sb = singles.tile([128, 1], F32, tag="bq")
            bk_sb = singles.tile([128, 1], F32, tag="bk")
            nc.sync.dma_start(out=bq_sb[:, :], in_=bq[:, :])
            nc.sync.dma_start(out=bk_sb[:, :], in_=bk[:, :])
            # b_V broadcast across all 128 partitions: [128, 128] f32
            bvrep_sb = singles.tile([128, 128], F32, tag="bvrep")
            bvap = bv[:, :]
            bv_src = bass.AP(
                tensor=bvap.tensor,
                offset=bvap.offset,
                ap=[[0, 128]] + list(bvap.ap[1:]),
            )
            nc.sync.dma_start(out=bvrep_sb[:, :], in_=bv_src)

            ident_sb = singles.tile([128, 128], BF16, tag="ident")
            mask_sb = singles.tile([128, 128], BF16, tag="mask")
            nc.sync.dma_start(out=ident_sb[:, :], in_=ident_dram[:, :])
            nc.sync.dma_start(out=mask_sb[:, :], in_=mask_dram[:, :])

            xt_sb = [
                [
                    singles.tile(
                        [128, SW], BF16, tag=f"xt{c}_{s}", name=f"xt_sb{c}_{s}"
                    )
                    for s in range(NS)
                ]
                for c in range(NKC)
            ]
            for s in range(NS):
                for c in range(NKC):
                    nc.sync.dma_start(
                        out=xt_sb[c][s][:, :],
                        in_=xt[c * 128 : (c + 1) * 128, s * SW : (s + 1) * SW],
                    )

            qt_sb = [
                singles.tile([128, SW], BF16, tag=f"qt{s}", name=f"qt_sb{s}")
                for s in range(NS)
            ]
            kt_sb = [
                singles.tile([128, SW], BF16, tag=f"kt{s}", name=f"kt_sb{s}")
                for s in range(NS)
            ]
            # V_aug pair per key block: [128(t), 130]; cols 0:64 head A,
            # col 64 ones(A), cols 65:129 head B, col 129 ones(B)
            va_sb = [
                singles.tile([128, 2 * (HD + 1)], BF16, tag=f"va{t}", name=f"va_sb{t}")
                for t in range(NTT)
            ]
            for t in range(NTT):
                nc.vector.memset(va_sb[t][:, HD : HD + 1], 1.0)
                nc.vector.memset(va_sb[t][:, 2 * HD + 1 : 2 * HD + 2], 1.0)

            # preload the exp table set while DMAs stream in
            warm_sb = singles.tile([1, 1], BF16, tag="warm")
            nc.scalar.activation(
                out=warm_sb[:, :],
                in_=bq_sb[0:1, 0:1],
                func=mybir.ActivationFunctionType.Exp,
                scale=0.125,
            )

            hrows = (slice(0, HD), slice(HD, 128))

            # ---------- emit helpers ----------
            def emit_proj_qk(s):
                sg = ps.tile([128, 2 * SW], F32, tag="sg", name="ps_qk")
                for idx, (w_sb, b_sb, dst) in enumerate(
                    ((wq_sb, bq_sb, qt_sb[s]), (wk_sb, bk_sb, kt_sb[s]))
                ):
                    off = idx * SW
                    for c in range(NKC):
                        nc.tensor.matmul(
                            sg[:, off : off + SW],
                            lhsT=w_sb[:, c, :],
                            rhs=xt_sb[c][s][:, :],
                            start=(c == 0),
                            stop=(c == NKC - 1),
                            skip_group_check=True,
                        )
                    nc.vector.tensor_scalar_add(
                        dst[:, :], sg[:, off : off + SW], b_sb[:, :]
                    )

            def emit_proj_v(s):
                # one full PSUM bank per t-tile: a DVE evac of one t-tile may
                # run while PE still writes another, and PE-write + DVE-read
                # in the SAME bank is a fatal collision
                for half in range(2):
                    sg = ps.tile([128, 2 * SW], F32, tag="sg", name="ps_v")
                    for tt in (2 * half, 2 * half + 1):
                        t = 4 * s + tt
                        off = (tt % 2) * SW
                        for c in range(NKC):
                            nc.tensor.matmul(
                                sg[:, off : off + 128],
                                lhsT=xt_sb[c][s][:, tt * 128 : (tt + 1) * 128],
                                rhs=wv_sb[:, c, :],
                                start=(c == 0),
                                stop=(c == NKC - 1),
                                skip_group_check=True,
                            )
                        # evac + b_V add in one op: dst [128,2,64] strided
                        # (skip the ones columns), src/bias [128,2,64]
                        dst3 = va_sb[t][:, 0 : 2 * (HD + 1)].rearrange(
                            "p (a b) -> p a b", a=2
                        )[:, :, 0:HD]
                        src3 = sg[:, off : off + 128].rearrange(
                            "p (a b) -> p a b", a=2
                        )
                        bv3 = bvrep_sb[:, :].rearrange("p (a b) -> p a b", a=2)
                        nc.vector.tensor_add(dst3, src3, bv3)

            def emit_scores(s, unit):
                kb, n, qlo, _pt = unit
                qs = s * SW
                diag = kb * 128 >= qs
                sg = ps.tile([128, 2 * SW], F32, tag="sg", name="ps_sg")
                unit[3] = sg
                for h in range(2):
                    off = h * SW
                    nc.tensor.matmul(
                        sg[:, off : off + n],
                        lhsT=kt_sb[kb // 4][
                            hrows[h], (kb % 4) * 128 : (kb % 4 + 1) * 128
                        ],
                        rhs=qt_sb[s][hrows[h], qlo - qs : qlo - qs + n],
                        start=True,
                        stop=not diag,
                        skip_group_check=True,
                        tile_position=(h * HD, 0),
                    )
                if diag:
                    for h in range(2):
                        nc.tensor.matmul(
                            sg[:, h * SW : h * SW + 128],
                            lhsT=ident_sb[:, :],
                            rhs=mask_sb[:, :],
                            start=False,
                            stop=True,
                            skip_group_check=True,
                        )

            def emit_exp(unit):
                kb, n, qlo, sg = unit
                pt = ptp.tile([128, 2 * SW], BF16, tag="pt", name="pt")
                in3 = sg[:, :].rearrange("p (a b) -> p a b", a=2)[:, :, 0:n]
                out3 = pt[:, 0 : 2 * n].rearrange("p (a b) -> p a b", a=2)
                nc.scalar.activation(
                    out=out3,
                    in_=in3,
                    func=mybir.ActivationFunctionType.Exp,
                    scale=0.125,
                )
                unit[3] = (pt, n)

            def emit_pv(s, unit, zaug):
                kb, n, qlo, (pt, ptn) = unit
                qs = s * SW
                nkb = 4 * (s + 1)
                for h in range(2):
                    nc.tensor.matmul(
                        zaug[h][0 : HD + 1, qlo - qs : SW],
                        lhsT=va_sb[kb][:, h * (HD + 1) : (h + 1) * (HD + 1)],
                        rhs=pt[:, h * ptn : h * ptn + n],
                        start=(kb == 0),
                        stop=(kb == nkb - 1),
                        skip_group_check=True,
                    )

            zsb_tiles = [None, None]

            def emit_zevac(zaug):
                for h in range(2):
                    t = slp.tile([HD + 1, SW], F32, tag=f"zsb{h}", name="zsb")
                    nc.vector.tensor_copy(t[:, :], zaug[h][0 : HD + 1, :])
                    zsb_tiles[h] = t

            pending_oproj = []

            def emit_norm(sp):
                znpair = znp.tile([128, SW], BF16, tag="zn")
                znb = slp.tile([HD, SW], BF16, tag="znb")
                for h in range(2):
                    zsb_h = zsb_tiles[h]
                    rd = drp.tile([1, SW], F32, tag=f"rd{h}", name="rd")
                    nc.sync.dma_start(out=rd[:, :], in_=zsb_h[HD : HD + 1, :])
                    lsp = slp.tile([128, SW // 128], F32, tag=f"lsp{h}", name="lsp")
                    nc.sync.dma_start(
                        out=lsp[:, :],
                        in_=rd[0, :].rearrange("(p f) -> p f", p=128),
                    )
                    rsp = slp.tile([128, SW // 128], F32, tag=f"rsp{h}", name="rsp")
                    nc.vector.reciprocal(rsp[:, :], lsp[:, :])
                    rd2 = drp.tile([1, SW], F32, tag=f"rd2{h}", name="rd2")
                    nc.sync.dma_start(
                        out=rd2[0, :].rearrange("(p f) -> p f", p=128),
                        in_=rsp[:, :],
                    )
                    bc = slp.tile([HD, SW], F32, tag=f"bc{h}")
                    rap = rd2[:, :]
                    bcast_src = bass.AP(
                        tensor=rap.tensor,
                        offset=rap.offset,
                        ap=[[0, HD]] + list(rap.ap[1:]),
                    )
                    nc.sync.dma_start(out=bc[:, :], in_=bcast_src)
                    dst = znpair[0:HD, :] if h == 0 else znb[:, :]
                    nc.vector.tensor_mul(dst, zsb_h[0:HD, :], bc[:, :])
                nc.gpsimd.dma_start(out=znpair[HD:128, :], in_=znb[:, :])
                pending_oproj.append((znpair, sp * SW))

            def emit_oproj_half(znpair_t, qs_t, jpair):
                op = ps.tile([128, 2 * SW], F32, tag="sg", name="ps_o")
                for jj in range(2):
                    j = jpair * 2 + jj
                    off = jj * SW
                    nc.tensor.matmul(
                        op[:, off : off + SW],
                        lhsT=znpair_t[:, j * 128 : (j + 1) * 128],
                        rhs=wo_sb[:, :],
                        start=True,
                        stop=True,
                        skip_group_check=True,
                    )
                    o_sb = outp.tile([128, D], F32, tag="ot", name="o_sb")
                    nc.vector.tensor_copy(o_sb[:, :], op[:, off : off + SW])
                    r0 = qs_t + j * 128
                    nc.sync.dma_start(out=out[r0 : r0 + 128, :], in_=o_sb[:, :])

            def emit_oproj(znpair_t, qs_t):
                emit_oproj_half(znpair_t, qs_t, 0)
                emit_oproj_half(znpair_t, qs_t, 1)

            # ---------- main loop ----------
            emit_proj_qk(0)
            emit_proj_v(0)
            prev_zaug = None
            for s in range(NS):
                qs = s * SW
                nkb = 4 * (s + 1)
                units = []
                for kb in range(nkb):
                    qlo = max(qs, kb * 128)
                    units.append([kb, qs + SW - qlo, qlo, None])
                L = len(units)

                # evacuate previous slice's accumulators first (DVE only)
                if prev_zaug is not None:
                    emit_zevac(prev_zaug)

                zaug = [
                    zps.tile([HD + 1, SW], F32, tag="za", name="zauga"),
                    zps.tile([HD + 1, SW], F32, tag="zb", name="zaugb"),
                ]

                def inserts(i, s=s, L=L):
                    if i == 2 and s >= 2 and pending_oproj:
                        emit_oproj(*pending_oproj.pop(0))
                    if i == 4 and s >= 1:
                        emit_norm(s - 1)
                    if i == min(6, L - 2) and s + 1 < NS:
                        emit_proj_qk(s + 1)
                        emit_proj_v(s + 1)
                    if i == L - 2 and s == NS - 1 and pending_oproj:
                        emit_oproj(*pending_oproj.pop(0))

                emit_scores(s, units[0])
                if L > 1:
                    emit_scores(s, units[1])
                for i in range(L):
                    emit_exp(units[i])
                    if i + 2 < L:
                        emit_scores(s, units[i + 2])
                    if i >= 1:
                        emit_pv(s, units[i - 1], zaug)
                    inserts(i)
                emit_pv(s, units[L - 1], zaug)
                prev_zaug = zaug

            # tail: last slice's evac, norm and O-projections
            emit_zevac(prev_zaug)
            emit_norm(NS - 1)
            for p in pending_oproj:
                emit_oproj(*p)
            pending_oproj.clear()

    _split_waits(nc)
    return nc


_NC_CACHE = {}


def _get_nc():
    if "nc" not in _NC_CACHE:
        _NC_CACHE["nc"] = build_nc()
    return _NC_CACHE["nc"]


def make_in_maps(combined_embed, W_K, b_K, W_Q, b_Q, W_V, b_V, W_O, b_O):
    f32 = np.float32
    in_maps = []
    for c in range(8):
        b = c // 4
        g = c % 4
        sl = slice(g * 128, (g + 1) * 128)
        xt = np.ascontiguousarray(np.asarray(combined_embed[b], f32).T)
        in_maps.append(
            {
                "xt": xt.astype(_BF16),
                "wq": np.ascontiguousarray(np.asarray(W_Q, f32)[:, sl]).astype(_BF16),
                "wk": np.ascontiguousarray(np.asarray(W_K, f32)[:, sl]).astype(_BF16),
                "wv": np.ascontiguousarray(np.asarray(W_V, f32)[:, sl]).astype(_BF16),
                "wo": np.ascontiguousarray(np.asarray(W_O, f32)[sl, :]).astype(_BF16),
                "bq": np.asarray(b_Q, f32)[sl].reshape(128, 1).copy(),
                "bk": np.asarray(b_K, f32)[sl].reshape(128, 1).copy(),
                "bv": np.asarray(b_V, f32)[sl].reshape(1, 128).copy(),
            }
        )
    return in_maps


def run_cores(in_maps, **kwargs):
    nc = _get_nc()
    return run_bass_kernel_spmd(nc, in_maps, core_ids=list(range(8)), **kwargs)


def kernel(
    combined_embed, W_K, b_K, W_Q, b_Q, W_V, b_V, W_O, b_O
):  # full inputs -> full output
    in_maps = make_in_maps(
        combined_embed, W_K, b_K, W_Q, b_Q, W_V, b_V, W_O, b_O
    )
    res = run_cores(in_maps)
    out = np.zeros((B, T, D), np.float32)
    for c in range(8):
        out[c // 4] += res.results[c]["out"]
    out += np.asarray(b_O, np.float32)[None, None, :]
    return out


# revision 11
# speedup vs baseline: 1.2497x; 1.0095x over previous
"""Multi-head causal self-attention on 8 TRN2 NeuronCores.

Problem: B=2, T=4096, D=512, H=8 heads (hd=64), fp32 in/out.

Sharding: core c in 0..7 handles batch b = c//4 and head pair g = c%4
(heads 2g, 2g+1 -> D-slice [128g, 128g+128)). Each core computes
    partial_out = concat_h( softmax(causal(Q_h K_h^T / 8)) V_h ) @ W_O[slice]
for its two heads; the host sums the 4 partials per batch and adds b_O.

Pipeline design (v3). ScalarE exp() is the hard floor (~17M causal score
elements per core at 1 elem/cycle/lane @1.2GHz) and TensorE is a close
second, so everything is organised to keep both streams dense and warm
(the HAM clock gate halves the PE clock after ~3.4us of idle):

  - Score slot = ONE key block x BOTH heads in one [128,1024] PSUM tile
    (head A bank 0, head B bank 1), one exp ACTIVATE per slot through a
    [128,2,n] strided AP -> 144 calls total.
  - One GLOBAL unit stream across all q-slices: scores run 2 units ahead
    of exp and PV runs 1 unit behind, ACROSS slice boundaries, so the
    exp stream never waits at a boundary.
  - QKV projection (slice s+1) and O-projection (slice s-2) are emitted
    MID-slice as extra PSUM-pool rotation slots.
  - Normalisation happens AFTER the O-projection: each head's O-proj is
    a separate 64-contraction matmul (row-tiled, concurrent), and the
    PSUM->SBUF evac scales by 1/L per output row (per-partition scalar)
    and fuses the two heads' add:  o = ps_A * (1/L_A) + ps_B * (1/L_B).
    1/L reaches [128,1] partition form via one DRAM-bounce spread per
    head per slice; no broadcast round-trip, no Z normalisation pass.
  - V bias is a DVE broadcast-add fused into the V PSUM->SBUF evac; each
    V t-tile owns a full PSUM bank (PE-write + DVE-read of the same bank
    is a fatal collision).
  - exp table set is preloaded with a dummy call during the DMA phase;
    input DMAs are ordered so the slice-0 projection starts ASAP.
"""

import numpy as np

import concourse.bass as bass
import concourse.mybir as mybir
from concourse.tile import TileContext
from concourse.bass_utils import run_bass_kernel_spmd

try:
    import ml_dtypes

    _BF16 = ml_dtypes.bfloat16
except ImportError:  # pragma: no cover
    _BF16 = None

F32 = mybir.dt.float32
BF16 = mybir.dt.bfloat16

B, T, D, H = 2, 4096, 512, 8
HD = D // H  # 64
SW = 512  # q-slice width
NS = T // SW  # 8 q-slices
NKC = D // 128  # 4 contraction chunks for the projections
NTT = T // 128  # 32 t-tiles / key blocks
NEG = -1.0e9


def _split_waits(nc, max_waits=1):
    """The staged walrus rejects >1 semaphore wait per instruction; hoist
    extras onto same-engine NoOps inserted right before the instruction."""
    counter = 0
    for f in nc.m.functions:
        for blk in f.blocks:
            insts = blk.instructions
            out, changed = [], False
            for ins in insts:
                si = getattr(ins, "sync_info", None)
                waits = list(si.on_wait) if si is not None and si.on_wait else []
                if len(waits) > max_waits:
                    changed = True
                    for w in waits[:-max_waits]:
                        counter += 1
                        nop = mybir.InstNoOp(
                            name=f"I-wsplit-{counter}",
                            engine=ins.engine,
                            ins=[],
                            outs=[],
                        )
                        nop.sync_info = mybir.SyncInfo(on_wait=[w], on_update=[])
                        out.append(nop)
                    ins.sync_info = mybir.SyncInfo(
                        on_wait=waits[-max_waits:], on_update=list(si.on_update)
                    )
                out.append(ins)
            if changed:
                blk.instructions = out
    return counter


def build_nc():
    nc = bass.Bass("TRN2")

    xt = nc.dram_tensor("xt", [D, T], BF16, kind="ExternalInput")
    wq = nc.dram_tensor("wq", [D, 128], BF16, kind="ExternalInput")
    wk = nc.dram_tensor("wk", [D, 128], BF16, kind="ExternalInput")
    wv = nc.dram_tensor("wv", [D, 128], BF16, kind="ExternalInput")
    wo = nc.dram_tensor("wo", [128, D], BF16, kind="ExternalInput")
    bq = nc.dram_tensor("bq", [128, 1], F32, kind="ExternalInput")
    bk = nc.dram_tensor("bk", [128, 1], F32, kind="ExternalInput")
    bv = nc.dram_tensor("bv", [1, 128], F32, kind="ExternalInput")
    out = nc.dram_tensor("out", [T, D], F32, kind="ExternalOutput")

    # maskneg[k, q'] = 0 where q' >= k else NEG  (S^T diagonal subtile mask)
    ident_np = np.eye(128, dtype=np.float32)
    mask_np = np.where(
        np.arange(128)[None, :] >= np.arange(128)[:, None], 0.0, NEG
    ).astype(np.float32)
    ident_dram = nc.inline_tensor(ident_np.astype(_BF16), name="identc")
    mask_dram = nc.inline_tensor(mask_np.astype(_BF16), name="maskc")

    with TileContext(nc) as tc:
        with (
            tc.tile_pool(name="singles", bufs=1) as singles,
            tc.tile_pool(name="ps", bufs=3, space="PSUM") as ps,
            tc.tile_pool(name="zps", bufs=1, space="PSUM") as zps,
            tc.tile_pool(name="pt", bufs=6) as ptp,
            tc.tile_pool(name="sl", bufs=2) as slp,
            tc.tile_pool(name="zn", bufs=3) as znp,
            tc.tile_pool(name="outp", bufs=4) as outp,
            tc.tile_pool(name="drp", bufs=2, space="DRAM") as drp,
        ):
            # ---- static SBUF; DMA order matters: slice-0 projection inputs
            # first so PE/ScalarE start ASAP ----
            bq_sb = singles.tile([128, 1], F32, tag="bq")
            nc.sync.dma_start(out=bq_sb[:, :], in_=bq[:, :])
            # preload the exp table set while the rest of the DMAs stream in
            warm_sb = singles.tile([1, 1], BF16, tag="warm")
            nc.scalar.activation(
                out=warm_sb[:, :],
                in_=bq_sb[0:1, 0:1],
                func=mybir.ActivationFunctionType.Exp,
                scale=0.125,
            )

            wq_sb = singles.tile([128, NKC, 128], BF16, tag="wq")
            wk_sb = singles.tile([128, NKC, 128], BF16, tag="wk")
            wv_sb = singles.tile([128, NKC, 128], BF16, tag="wv")
            for c in range(NKC):
                nc.sync.dma_start(out=wq_sb[:, c, :], in_=wq[c * 128 : (c + 1) * 128, :])
            xt_sb = [
                [
                    singles.tile(
                        [128, SW], BF16, tag=f"xt{c}_{s}", name=f"xt_sb{c}_{s}"
                    )
                    for s in range(NS)
                ]
                for c in range(NKC)
            ]
            for c in range(NKC):
                nc.sync.dma_start(
                    out=xt_sb[c][0][:, :], in_=xt[c * 128 : (c + 1) * 128, 0:SW]
                )
            bk_sb = singles.tile([128, 1], F32, tag="bk")
            nc.sync.dma_start(out=bk_sb[:, :], in_=bk[:, :])
            for c in range(NKC):
                nc.sync.dma_start(out=wk_sb[:, c, :], in_=wk[c * 128 : (c + 1) * 128, :])
            ident_sb = singles.tile([128, 128], BF16, tag="ident")
            mask_sb = singles.tile([128, 128], BF16, tag="mask")
            nc.sync.dma_start(out=ident_sb[:, :], in_=ident_dram[:, :])
            nc.sync.dma_start(out=mask_sb[:, :], in_=mask_dram[:, :])
            for c in range(NKC):
                nc.sync.dma_start(out=wv_sb[:, c, :], in_=wv[c * 128 : (c + 1) * 128, :])
            # b_V broadcast across all 128 partitions: [128, 128] f32
            bvrep_sb = singles.tile([128, 128], F32, tag="bvrep")
            bvap = bv[:, :]
            bv_src = bass.AP(
                tensor=bvap.tensor,
                offset=bvap.offset,
                ap=[[0, 128]] + list(bvap.ap[1:]),
            )
            nc.sync.dma_start(out=bvrep_sb[:, :], in_=bv_src)
            wo_sb = singles.tile([128, D], BF16, tag="wo")
            nc.sync.dma_start(out=wo_sb[:, :], in_=wo[:, :])
            for s in range(1, NS):
                for c in range(NKC):
                    nc.sync.dma_start(
                        out=xt_sb[c][s][:, :],
                        in_=xt[c * 128 : (c + 1) * 128, s * SW : (s + 1) * SW],
                    )

            qt_sb = [
                singles.tile([128, SW], BF16, tag=f"qt{s}", name=f"qt_sb{s}")
                for s in range(NS)
            ]
            kt_sb = [
                singles.tile([128, SW], BF16, tag=f"kt{s}", name=f"kt_sb{s}")
                for s in range(NS)
            ]
            # V_aug pair per key block: [128(t), 130]; cols 0:64 head A,
            # col 64 ones(A), cols 65:129 head B, col 129 ones(B)
            va_sb = [
                singles.tile([128, 2 * (HD + 1)], BF16, tag=f"va{t}", name=f"va_sb{t}")
                for t in range(NTT)
            ]
            for t in range(NTT):
                nc.vector.memset(va_sb[t][:, HD : HD + 1], 1.0)
                nc.vector.memset(va_sb[t][:, 2 * HD + 1 : 2 * HD + 2], 1.0)

            hrows = (slice(0, HD), slice(HD, 128))

            # ---------- emit helpers ----------
            def emit_proj_qk(s):
                sg = ps.tile([128, 2 * SW], F32, tag="sg", name="ps_qk")
                for idx, (w_sb, b_sb, dst) in enumerate(
                    ((wq_sb, bq_sb, qt_sb[s]), (wk_sb, bk_sb, kt_sb[s]))
                ):
                    off = idx * SW
                    for c in range(NKC):
                        nc.tensor.matmul(
                            sg[:, off : off + SW],
                            lhsT=w_sb[:, c, :],
                            rhs=xt_sb[c][s][:, :],
                            start=(c == 0),
                            stop=(c == NKC - 1),
                            skip_group_check=True,
                        )
                    nc.vector.tensor_scalar_add(
                        dst[:, :], sg[:, off : off + SW], b_sb[:, :]
                    )

            def emit_proj_v(s):
                # one full PSUM bank per t-tile: a DVE evac of one t-tile may
                # run while PE still writes another, and PE-write + DVE-read
                # in the SAME bank is a fatal collision
                for half in range(2):
                    sg = ps.tile([128, 2 * SW], F32, tag="sg", name="ps_v")
                    for tt in (2 * half, 2 * half + 1):
                        t = 4 * s + tt
                        off = (tt % 2) * SW
                        for c in range(NKC):
                            nc.tensor.matmul(
                                sg[:, off : off + 128],
                                lhsT=xt_sb[c][s][:, tt * 128 : (tt + 1) * 128],
                                rhs=wv_sb[:, c, :],
                                start=(c == 0),
                                stop=(c == NKC - 1),
                                skip_group_check=True,
                            )
                        # evac + b_V add in one op: dst [128,2,64] strided
                        # (skip the ones columns), src/bias [128,2,64]
                        dst3 = va_sb[t][:, 0 : 2 * (HD + 1)].rearrange(
                            "p (a b) -> p a b", a=2
                        )[:, :, 0:HD]
                        src3 = sg[:, off : off + 128].rearrange(
                            "p (a b) -> p a b", a=2
                        )
                        bv3 = bvrep_sb[:, :].rearrange("p (a b) -> p a b", a=2)
                        nc.vector.tensor_add(dst3, src3, bv3)

            def emit_scores(unit):
                s, kb, n, qlo = unit[:4]
                qs = s * SW
                diag = kb * 128 >= qs
                sg = ps.tile([128, 2 * SW], F32, tag="sg", name="ps_sg")
                unit[4] = sg
                for h in range(2):
                    off = h * SW
                    nc.tensor.matmul(
                        sg[:, off : off + n],
                        lhsT=kt_sb[kb // 4][
                            hrows[h], (kb % 4) * 128 : (kb % 4 + 1) * 128
                        ],
                        rhs=qt_sb[s][hrows[h], qlo - qs : qlo - qs + n],
                        start=True,
                        stop=not diag,
                        skip_group_check=True,
                        tile_position=(h * HD, 0),
                    )
                if diag:
                    for h in range(2):
                        nc.tensor.matmul(
                            sg[:, h * SW : h * SW + 128],
                            lhsT=ident_sb[:, :],
                            rhs=mask_sb[:, :],
                            start=False,
                            stop=True,
                            skip_group_check=True,
                        )

            def emit_exp(unit):
                s, kb, n, qlo, sg = unit[:5]
                pt = ptp.tile([128, 2 * SW], BF16, tag="pt", name="pt")
                in3 = sg[:, :].rearrange("p (a b) -> p a b", a=2)[:, :, 0:n]
                out3 = pt[:, 0 : 2 * n].rearrange("p (a b) -> p a b", a=2)
                nc.scalar.activation(
                    out=out3,
                    in_=in3,
                    func=mybir.ActivationFunctionType.Exp,
                    scale=0.125,
                )
                unit[4] = pt

            def emit_pv(unit, zaug):
                s, kb, n, qlo, pt = unit[:5]
                qs = s * SW
                nkb = 4 * (s + 1)
                for h in range(2):
                    nc.tensor.matmul(
                        zaug[h][0 : HD + 1, qlo - qs : SW],
                        lhsT=va_sb[kb][:, h * (HD + 1) : (h + 1) * (HD + 1)],
                        rhs=pt[:, h * n : h * n + n],
                        start=(kb == 0),
                        stop=(kb == nkb - 1),
                        skip_group_check=True,
                    )

            # O-proj work items: (zpair, ra, rb, qs, j, ready_at_iter).
            # Drained one q-tile per stream iteration once ready_at passes,
            # so the 1.3us of DVE evac per q-tile never bunches up.
            oproj_work = []

            def emit_zfinish(sp, zaug, g0=0, last=False):
                """Evacuate slice sp's Z/L accumulators and set up the
                deferred, post-O-proj normalisation: unnormalised Z pair
                stacked [128,512] bf16 (head B shifted to partitions
                64..127 via GpSimd) + per-q-tile [128,1] 1/L spreads."""
                zpair = znp.tile([128, SW], BF16, tag="zn")
                znb = slp.tile([HD, SW], BF16, tag="znb")
                nc.vector.tensor_copy(zpair[0:HD, :], zaug[0][0:HD, :])
                nc.vector.tensor_copy(znb[:, :], zaug[1][0:HD, :])
                rinv = []
                for h in range(2):
                    lr = slp.tile([1, SW], F32, tag=f"lr{h}", name="lrow")
                    if last:
                        # ScalarE is idle at the tail; take it off DVE
                        nc.scalar.copy(lr[:, :], zaug[h][HD : HD + 1, :])
                    else:
                        nc.vector.tensor_copy(lr[:, :], zaug[h][HD : HD + 1, :])
                    rd = drp.tile([1, SW], F32, tag=f"rd{h}", name="rd")
                    nc.sync.dma_start(out=rd[:, :], in_=lr[:, :])
                    # [1,512] -> [128,4]: partition p, col j = L[j*128+p]
                    li = znp.tile([128, SW // 128], F32, tag=f"li{h}", name="li")
                    nc.sync.dma_start(
                        out=li[:, :],
                        in_=rd[0, :].rearrange("(a p) -> p a", p=128),
                    )
                    ri = znp.tile([128, SW // 128], F32, tag=f"ri{h}", name="ri")
                    nc.vector.reciprocal(ri[:, :], li[:, :])
                    rinv.append(ri)
                nc.gpsimd.dma_start(out=zpair[HD:128, :], in_=znb[:, :])
                for j in range(4):
                    oproj_work.append(
                        (zpair, rinv[0], rinv[1], sp * SW, j, g0 + 4 + j)
                    )

            def emit_oproj_qtile(item, scalar_assist=False):
                zpair, ra, rb, qs_t, j, _ = item
                op = ps.tile([128, 2 * SW], F32, tag="sg", name="ps_o")
                jq = slice(j * 128, (j + 1) * 128)
                nc.tensor.matmul(
                    op[:, 0:SW],
                    lhsT=zpair[0:HD, jq],
                    rhs=wo_sb[0:HD, :],
                    start=True,
                    stop=True,
                    skip_group_check=True,
                    tile_position=(0, 0),
                )
                nc.tensor.matmul(
                    op[:, SW : 2 * SW],
                    lhsT=zpair[HD:128, jq],
                    rhs=wo_sb[HD:128, :],
                    start=True,
                    stop=True,
                    skip_group_check=True,
                    tile_position=(HD, 0),
                )
                o_tmp = outp.tile([128, D], F32, tag="otmp", name="o_tmp")
                if scalar_assist:
                    nc.scalar.mul(o_tmp[:, :], op[:, SW : 2 * SW], rb[:, j : j + 1])
                else:
                    nc.vector.tensor_scalar_mul(
                        o_tmp[:, :], op[:, SW : 2 * SW], rb[:, j : j + 1]
                    )
                o_sb = outp.tile([128, D], F32, tag="ot", name="o_sb")
                nc.vector.scalar_tensor_tensor(
                    o_sb[:, :],
                    op[:, 0:SW],
                    ra[:, j : j + 1],
                    o_tmp[:, :],
                    op0=mybir.AluOpType.mult,
                    op1=mybir.AluOpType.add,
                )
                r0 = qs_t + j * 128
                nc.sync.dma_start(out=out[r0 : r0 + 128, :], in_=o_sb[:, :])

            # ---------- global unit stream ----------
            stream = []
            first_of_slice = {}
            for s in range(NS):
                qs = s * SW
                first_of_slice[s] = len(stream)
                for kb in range(4 * (s + 1)):
                    qlo = max(qs, kb * 128)
                    stream.append([s, kb, qs + SW - qlo, qlo, None])
            G = len(stream)

            def inserts(s, i, g):
                L = 4 * (s + 1)
                if s + 1 < NS:
                    if i == min(2, L - 2) and s == 0:
                        emit_proj_qk(s + 1)
                        emit_proj_v(s + 1)
                    elif i == 6 and s >= 1:
                        emit_proj_qk(s + 1)
                    elif i == 7 and s >= 1:
                        emit_proj_v(s + 1)
                if oproj_work and g >= oproj_work[0][5]:
                    emit_oproj_qtile(oproj_work.pop(0))

            emit_proj_qk(0)
            emit_proj_v(0)
            emit_scores(stream[0])
            emit_scores(stream[1])
            zaug = None
            prev_zaug = None
            for g in range(G):
                s, kb = stream[g][0], stream[g][1]
                emit_exp(stream[g])
                inserts(s, g - first_of_slice[s], g)
                if g + 2 < G:
                    emit_scores(stream[g + 2])
                if g >= 1:
                    ps_, pkb = stream[g - 1][0], stream[g - 1][1]
                    if pkb == 0:
                        # stream[g-1] opens slice ps_: finish the previous
                        # slice's accumulators, then claim fresh ones
                        if prev_zaug is not None:
                            emit_zfinish(ps_ - 1, prev_zaug, g0=g)
                        zaug = [
                            zps.tile([HD + 1, SW], F32, tag="za", name="zauga"),
                            zps.tile([HD + 1, SW], F32, tag="zb", name="zaugb"),
                        ]
                        prev_zaug = zaug
                    emit_pv(stream[g - 1], zaug)
            emit_pv(stream[G - 1], zaug)

            # tail: last slice's Z finish + remaining O-projections
            emit_zfinish(NS - 1, zaug, g0=G, last=True)
            while oproj_work:
                emit_oproj_qtile(oproj_work.pop(0), scalar_assist=True)

    _split_waits(nc)
    return nc


_NC_CACHE = {}


def _get_nc():
    if "nc" not in _NC_CACHE:
        _NC_CACHE["nc"] = build_nc()
    return _NC_CACHE["nc"]


def make_in_maps(combined_embed, W_K, b_K, W_Q, b_Q, W_V, b_V, W_O, b_O):
    f32 = np.float32
    in_maps = []
    for c in range(8):
        b = c // 4
        g = c % 4
        sl = slice(g * 128, (g + 1) * 128)
        xt = np.ascontiguousarray(np.asarray(combined_embed[b], f32).T)
        in_maps.append(
            {
                "xt": xt.astype(_BF16),
                "wq": np.ascontiguousarray(np.asarray(W_Q, f32)[:, sl]).astype(_BF16),
                "wk": np.ascontiguousarray(np.asarray(W_K, f32)[:, sl]).astype(_BF16),
                "wv": np.ascontiguousarray(np.asarray(W_V, f32)[:, sl]).astype(_BF16),
                "wo": np.ascontiguousarray(np.asarray(W_O, f32)[sl, :]).astype(_BF16),
                "bq": np.asarray(b_Q, f32)[sl].reshape(128, 1).copy(),
                "bk": np.asarray(b_K, f32)[sl].reshape(128, 1).copy(),
                "bv": np.asarray(b_V, f32)[sl].reshape(1, 128).copy(),
            }
        )
    return in_maps


def run_cores(in_maps, **kwargs):
    nc = _get_nc()
    return run_bass_kernel_spmd(nc, in_maps, core_ids=list(range(8)), **kwargs)


def kernel(
    combined_embed, W_K, b_K, W_Q, b_Q, W_V, b_V, W_O, b_O
):  # full inputs -> full output
    in_maps = make_in_maps(
        combined_embed, W_K, b_K, W_Q, b_Q, W_V, b_V, W_O, b_O
    )
    res = run_cores(in_maps)
    out = np.zeros((B, T, D), np.float32)
    for c in range(8):
        out[c // 4] += res.results[c]["out"]
    out += np.asarray(b_O, np.float32)[None, None, :]
    return out


# revision 16
# speedup vs baseline: 1.3115x; 1.0495x over previous
"""Multi-head causal self-attention on 8 TRN2 NeuronCores.

Problem: B=2, T=4096, D=512, H=8 heads (hd=64), fp32 in/out.

Sharding: core c in 0..7 handles batch b = c//4 and head pair g = c%4
(heads 2g, 2g+1 -> D-slice [128g, 128g+128)). Each core computes
    partial_out = concat_h( softmax(causal(Q_h K_h^T / 8)) V_h ) @ W_O[slice]
for its two heads; the host sums the 4 partials per batch and adds b_O.

Pipeline design (v4). ScalarE exp() is the hard floor (~17M causal score
elements per core at 1 elem/cycle/lane @1.2GHz) and TensorE is a close
second, so everything is organised to keep both streams dense and warm
(the HAM clock gate halves the PE clock after ~3.4us of idle):

  - Score slot = ONE key block x BOTH heads in one [128,1024] PSUM tile
    (head A bank 0, head B bank 1), one exp ACTIVATE per slot through a
    [128,2,n] strided AP -> 144 calls total.
  - One GLOBAL unit stream across all q-slices: scores run 1 unit ahead
    of exp and PV runs 1 unit behind, ACROSS slice boundaries, so the
    exp stream never waits at a boundary.
  - QKV projection (slice s+1) and O-projection slots live in their OWN
    2-bank PSUM pool ("ex"), so they never steal slots from the
    exp-paced score rotation (which would stall the exp stream by one
    exp period per stolen slot).
  - Normalisation happens AFTER the O-projection: each head's O-proj is
    a separate 64-contraction matmul (row-tiled, concurrent), and the
    PSUM->SBUF evac scales by 1/L per output row (per-partition scalar)
    and fuses the two heads' add:  o = ps_A * (1/L_A) + ps_B * (1/L_B).
    1/L reaches [128,1] partition form via one DRAM-bounce spread per
    head per slice; no broadcast round-trip, no Z normalisation pass.
  - V bias is a DVE broadcast-add fused into the V PSUM->SBUF evac; each
    V t-tile owns a full PSUM bank (PE-write + DVE-read of the same bank
    is a fatal collision).
  - exp table set is preloaded with a dummy call during the DMA phase;
    input DMAs are ordered so the slice-0 projection starts ASAP.
"""

import numpy as np

import concourse.bass as bass
import concourse.mybir as mybir
from concourse.tile import TileContext
from concourse.bass_utils import run_bass_kernel_spmd

try:
    import ml_dtypes

    _BF16 = ml_dtypes.bfloat16
except ImportError:  # pragma: no cover
    _BF16 = None

F32 = mybir.dt.float32
BF16 = mybir.dt.bfloat16

B, T, D, H = 2, 4096, 512, 8
HD = D // H  # 64
SW = 512  # q-slice width
NS = T // SW  # 8 q-slices
NKC = D // 128  # 4 contraction chunks for the projections
NTT = T // 128  # 32 t-tiles / key blocks
NEG = -1.0e9


def _split_waits(nc, max_waits=1):
    """The staged walrus rejects >1 semaphore wait per instruction; hoist
    extras onto same-engine NoOps inserted right before the instruction."""
    counter = 0
    for f in nc.m.functions:
        for blk in f.blocks:
            insts = blk.instructions
            out, changed = [], False
            for ins in insts:
                si = getattr(ins, "sync_info", None)
                waits = list(si.on_wait) if si is not None and si.on_wait else []
                if len(waits) > max_waits:
                    changed = True
                    for w in waits[:-max_waits]:
                        counter += 1
                        nop = mybir.InstNoOp(
                            name=f"I-wsplit-{counter}",
                            engine=ins.engine,
                            ins=[],
                            outs=[],
                        )
                        nop.sync_info = mybir.SyncInfo(on_wait=[w], on_update=[])
                        out.append(nop)
                    ins.sync_info = mybir.SyncInfo(
                        on_wait=waits[-max_waits:], on_update=list(si.on_update)
                    )
                out.append(ins)
            if changed:
                blk.instructions = out
    return counter


def build_nc():
    nc = bass.Bass("TRN2")

    xt = nc.dram_tensor("xt", [D, T], BF16, kind="ExternalInput")
    wq = nc.dram_tensor("wq", [D, 128], BF16, kind="ExternalInput")
    wk = nc.dram_tensor("wk", [D, 128], BF16, kind="ExternalInput")
    wv = nc.dram_tensor("wv", [D, 128], BF16, kind="ExternalInput")
    wo = nc.dram_tensor("wo", [128, D], BF16, kind="ExternalInput")
    bq = nc.dram_tensor("bq", [128, 1], F32, kind="ExternalInput")
    bk = nc.dram_tensor("bk", [128, 1], F32, kind="ExternalInput")
    bv = nc.dram_tensor("bv", [1, 128], F32, kind="ExternalInput")
    out = nc.dram_tensor("out", [T, D], F32, kind="ExternalOutput")

    # maskneg[k, q'] = 0 where q' >= k else NEG  (S^T diagonal subtile mask)
    ident_np = np.eye(128, dtype=np.float32)
    mask_np = np.where(
        np.arange(128)[None, :] >= np.arange(128)[:, None], 0.0, NEG
    ).astype(np.float32)
    ident_dram = nc.inline_tensor(ident_np.astype(_BF16), name="identc")
    mask_dram = nc.inline_tensor(mask_np.astype(_BF16), name="maskc")

    with TileContext(nc) as tc:
        with (
            tc.tile_pool(name="singles", bufs=1) as singles,
            tc.tile_pool(name="ps", bufs=2, space="PSUM") as ps,
            tc.tile_pool(name="ex", bufs=1, space="PSUM") as ext,
            tc.tile_pool(name="zps", bufs=1, space="PSUM") as zps,
            tc.tile_pool(name="pt", bufs=6) as ptp,
            tc.tile_pool(name="sl", bufs=2) as slp,
            tc.tile_pool(name="zn", bufs=3) as znp,
            tc.tile_pool(name="outp", bufs=4) as outp,
            tc.tile_pool(name="drp", bufs=2, space="DRAM") as drp,
        ):
            # ---- static SBUF; DMA order matters: slice-0 projection inputs
            # first so PE/ScalarE start ASAP ----
            bq_sb = singles.tile([128, 1], F32, tag="bq")
            nc.sync.dma_start(out=bq_sb[:, :], in_=bq[:, :])
            # preload the exp table set while the rest of the DMAs stream in
            warm_sb = singles.tile([1, 1], BF16, tag="warm")
            nc.scalar.activation(
                out=warm_sb[:, :],
                in_=bq_sb[0:1, 0:1],
                func=mybir.ActivationFunctionType.Exp,
                scale=0.125,
            )

            wq_sb = singles.tile([128, NKC, 128], BF16, tag="wq")
            wk_sb = singles.tile([128, NKC, 128], BF16, tag="wk")
            wv_sb = singles.tile([128, NKC, 128], BF16, tag="wv")
            for c in range(NKC):
                nc.sync.dma_start(out=wq_sb[:, c, :], in_=wq[c * 128 : (c + 1) * 128, :])
            xt_sb = [
                [
                    singles.tile(
                        [128, SW], BF16, tag=f"xt{c}_{s}", name=f"xt_sb{c}_{s}"
                    )
                    for s in range(NS)
                ]
                for c in range(NKC)
            ]
            for c in range(NKC):
                nc.sync.dma_start(
                    out=xt_sb[c][0][:, :], in_=xt[c * 128 : (c + 1) * 128, 0:SW]
                )
            bk_sb = singles.tile([128, 1], F32, tag="bk")
            nc.sync.dma_start(out=bk_sb[:, :], in_=bk[:, :])
            for c in range(NKC):
                nc.sync.dma_start(out=wk_sb[:, c, :], in_=wk[c * 128 : (c + 1) * 128, :])
            ident_sb = singles.tile([128, 128], BF16, tag="ident")
            mask_sb = singles.tile([128, 128], BF16, tag="mask")
            nc.sync.dma_start(out=ident_sb[:, :], in_=ident_dram[:, :])
            nc.sync.dma_start(out=mask_sb[:, :], in_=mask_dram[:, :])
            for c in range(NKC):
                nc.sync.dma_start(out=wv_sb[:, c, :], in_=wv[c * 128 : (c + 1) * 128, :])
            # b_V broadcast across all 128 partitions: [128, 128] f32
            bvrep_sb = singles.tile([128, 128], F32, tag="bvrep")
            bvap = bv[:, :]
            bv_src = bass.AP(
                tensor=bvap.tensor,
                offset=bvap.offset,
                ap=[[0, 128]] + list(bvap.ap[1:]),
            )
            nc.sync.dma_start(out=bvrep_sb[:, :], in_=bv_src)
            wo_sb = singles.tile([128, D], BF16, tag="wo")
            nc.sync.dma_start(out=wo_sb[:, :], in_=wo[:, :])
            for s in range(1, NS):
                for c in range(NKC):
                    nc.sync.dma_start(
                        out=xt_sb[c][s][:, :],
                        in_=xt[c * 128 : (c + 1) * 128, s * SW : (s + 1) * SW],
                    )

            qt_sb = [
                singles.tile([128, SW], BF16, tag=f"qt{s}", name=f"qt_sb{s}")
                for s in range(NS)
            ]
            kt_sb = [
                singles.tile([128, SW], BF16, tag=f"kt{s}", name=f"kt_sb{s}")
                for s in range(NS)
            ]
            # V_aug pair per key block: [128(t), 130]; cols 0:64 head A,
            # col 64 ones(A), cols 65:129 head B, col 129 ones(B)
            va_sb = [
                singles.tile([128, 2 * (HD + 1)], BF16, tag=f"va{t}", name=f"va_sb{t}")
                for t in range(NTT)
            ]
            for t in range(NTT):
                nc.vector.memset(va_sb[t][:, HD : HD + 1], 1.0)
                nc.vector.memset(va_sb[t][:, 2 * HD + 1 : 2 * HD + 2], 1.0)

            hrows = (slice(0, HD), slice(HD, 128))

            # ---------- emit helpers ----------
            def emit_proj_qk(s):
                sg = ext.tile([128, 2 * SW], F32, tag="ex", name="ps_qk")
                for idx, (w_sb, b_sb, dst) in enumerate(
                    ((wq_sb, bq_sb, qt_sb[s]), (wk_sb, bk_sb, kt_sb[s]))
                ):
                    off = idx * SW
                    for c in range(NKC):
                        nc.tensor.matmul(
                            sg[:, off : off + SW],
                            lhsT=w_sb[:, c, :],
                            rhs=xt_sb[c][s][:, :],
                            start=(c == 0),
                            stop=(c == NKC - 1),
                            skip_group_check=True,
                        )
                    nc.vector.tensor_scalar_add(
                        dst[:, :], sg[:, off : off + SW], b_sb[:, :]
                    )

            def emit_proj_v_half(s, half):
                # one full PSUM bank per t-tile: a DVE evac of one t-tile may
                # run while PE still writes another, and PE-write + DVE-read
                # in the SAME bank is a fatal collision
                if True:
                    sg = ext.tile([128, 2 * SW], F32, tag="ex", name="ps_v")
                    for tt in (2 * half, 2 * half + 1):
                        t = 4 * s + tt
                        off = (tt % 2) * SW
                        for c in range(NKC):
                            nc.tensor.matmul(
                                sg[:, off : off + 128],
                                lhsT=xt_sb[c][s][:, tt * 128 : (tt + 1) * 128],
                                rhs=wv_sb[:, c, :],
                                start=(c == 0),
                                stop=(c == NKC - 1),
                                skip_group_check=True,
                            )
                        # evac + b_V add in one op: dst [128,2,64] strided
                        # (skip the ones columns), src/bias [128,2,64]
                        dst3 = va_sb[t][:, 0 : 2 * (HD + 1)].rearrange(
                            "p (a b) -> p a b", a=2
                        )[:, :, 0:HD]
                        src3 = sg[:, off : off + 128].rearrange(
                            "p (a b) -> p a b", a=2
                        )
                        bv3 = bvrep_sb[:, :].rearrange("p (a b) -> p a b", a=2)
                        nc.vector.tensor_add(dst3, src3, bv3)

            def emit_scores(unit):
                s, kb, n, qlo = unit[:4]
                qs = s * SW
                diag = kb * 128 >= qs
                sg = ps.tile([128, 2 * SW], F32, tag="sg", name="ps_sg")
                unit[4] = sg
                for h in range(2):
                    off = h * SW
                    nc.tensor.matmul(
                        sg[:, off : off + n],
                        lhsT=kt_sb[kb // 4][
                            hrows[h], (kb % 4) * 128 : (kb % 4 + 1) * 128
                        ],
                        rhs=qt_sb[s][hrows[h], qlo - qs : qlo - qs + n],
                        start=True,
                        stop=not diag,
                        skip_group_check=True,
                        tile_position=(h * HD, 0),
                    )
                if diag:
                    for h in range(2):
                        nc.tensor.matmul(
                            sg[:, h * SW : h * SW + 128],
                            lhsT=ident_sb[:, :],
                            rhs=mask_sb[:, :],
                            start=False,
                            stop=True,
                            skip_group_check=True,
                        )

            def emit_exp(unit):
                s, kb, n, qlo, sg = unit[:5]
                pt = ptp.tile([128, 2 * SW], BF16, tag="pt", name="pt")
                in3 = sg[:, :].rearrange("p (a b) -> p a b", a=2)[:, :, 0:n]
                out3 = pt[:, 0 : 2 * n].rearrange("p (a b) -> p a b", a=2)
                nc.scalar.activation(
                    out=out3,
                    in_=in3,
                    func=mybir.ActivationFunctionType.Exp,
                    scale=0.125,
                )
                unit[4] = pt

            def emit_pv(unit, zaug):
                s, kb, n, qlo, pt = unit[:5]
                qs = s * SW
                nkb = 4 * (s + 1)
                for h in range(2):
                    nc.tensor.matmul(
                        zaug[h][0 : HD + 1, qlo - qs : SW],
                        lhsT=va_sb[kb][:, h * (HD + 1) : (h + 1) * (HD + 1)],
                        rhs=pt[:, h * n : h * n + n],
                        start=(kb == 0),
                        stop=(kb == nkb - 1),
                        skip_group_check=True,
                    )

            # O-proj work items: (zpair, li_pair_or_ri_pair, qs, j, ready_at).
            # Drained one q-tile per stream iteration once ready_at passes,
            # so the 1.3us of DVE evac per q-tile never bunches up.  The
            # reciprocals are deferred to the first q-tile item (j==0): in
            # emit_zfinish they would block the DVE FIFO on the L-spread
            # DRAM round-trip.
            oproj_work = []

            def emit_zfinish(sp, zaug, g0=0, last=False):
                """Evacuate slice sp's Z/L accumulators and set up the
                deferred, post-O-proj normalisation: unnormalised Z pair
                stacked [128,512] bf16 (head B shifted to partitions
                64..127 via GpSimd) + per-q-tile [128,1] 1/L spreads."""
                zpair = znp.tile([128, SW], BF16, tag="zn")
                znb = slp.tile([HD, SW], BF16, tag="znb")
                nc.vector.tensor_copy(zpair[0:HD, :], zaug[0][0:HD, :])
                nc.vector.tensor_copy(znb[:, :], zaug[1][0:HD, :])
                lri = []
                for h in range(2):
                    lr = slp.tile([1, SW], F32, tag=f"lr{h}", name="lrow")
                    if last:
                        # ScalarE is idle at the tail; take it off DVE
                        nc.scalar.copy(lr[:, :], zaug[h][HD : HD + 1, :])
                    else:
                        nc.vector.tensor_copy(lr[:, :], zaug[h][HD : HD + 1, :])
                    rd = drp.tile([1, SW], F32, tag=f"rd{h}", name="rd")
                    nc.sync.dma_start(out=rd[:, :], in_=lr[:, :])
                    # [1,512] -> [128,4]: partition p, col j = L[j*128+p]
                    li = znp.tile([128, SW // 128], F32, tag=f"li{h}", name="li")
                    nc.sync.dma_start(
                        out=li[:, :],
                        in_=rd[0, :].rearrange("(a p) -> p a", p=128),
                    )
                    lri.append(li)
                nc.gpsimd.dma_start(out=zpair[HD:128, :], in_=znb[:, :])
                state = {"li": lri, "ri": None}
                for j in range(4):
                    oproj_work.append((zpair, state, sp * SW, j, g0 + 4 + j))

            def emit_oproj_qtile(item, scalar_assist=False):
                zpair, state, qs_t, j, _ = item
                if state["ri"] is None:
                    ri_pair = []
                    for h in range(2):
                        ri = znp.tile(
                            [128, SW // 128], F32, tag=f"ri{h}", name="ri"
                        )
                        nc.vector.reciprocal(ri[:, :], state["li"][h][:, :])
                        ri_pair.append(ri)
                    state["ri"] = ri_pair
                ra, rb = state["ri"]
                op = ext.tile([128, 2 * SW], F32, tag="ex", name="ps_o")
                jq = slice(j * 128, (j + 1) * 128)
                nc.tensor.matmul(
                    op[:, 0:SW],
                    lhsT=zpair[0:HD, jq],
                    rhs=wo_sb[0:HD, :],
                    start=True,
                    stop=True,
                    skip_group_check=True,
                    tile_position=(0, 0),
                )
                nc.tensor.matmul(
                    op[:, SW : 2 * SW],
                    lhsT=zpair[HD:128, jq],
                    rhs=wo_sb[HD:128, :],
                    start=True,
                    stop=True,
                    skip_group_check=True,
                    tile_position=(HD, 0),
                )
                o_tmp = outp.tile([128, D], F32, tag="otmp", name="o_tmp")
                if scalar_assist:
                    nc.scalar.mul(o_tmp[:, :], op[:, SW : 2 * SW], rb[:, j : j + 1])
                else:
                    nc.vector.tensor_scalar_mul(
                        o_tmp[:, :], op[:, SW : 2 * SW], rb[:, j : j + 1]
                    )
                o_sb = outp.tile([128, D], F32, tag="ot", name="o_sb")
                nc.vector.scalar_tensor_tensor(
                    o_sb[:, :],
                    op[:, 0:SW],
                    ra[:, j : j + 1],
                    o_tmp[:, :],
                    op0=mybir.AluOpType.mult,
                    op1=mybir.AluOpType.add,
                )
                r0 = qs_t + j * 128
                nc.sync.dma_start(out=out[r0 : r0 + 128, :], in_=o_sb[:, :])

            # ---------- global unit stream ----------
            stream = []
            first_of_slice = {}
            for s in range(NS):
                qs = s * SW
                first_of_slice[s] = len(stream)
                for kb in range(4 * (s + 1)):
                    qlo = max(qs, kb * 128)
                    stream.append([s, kb, qs + SW - qlo, qlo, None])
            G = len(stream)

            def inserts(s, i, g):
                # extras schedule (one PSUM "ex" slot each, at most ~1/iter):
                # proj for slice s+1 must be fully emitted before the scores
                # of s+1 u0 (iter L-1); V halves feed PV with plenty of lag
                if s == 0:
                    if i == 0:
                        emit_proj_v_half(0, 0)
                    elif i == 1:
                        emit_proj_v_half(0, 1)
                    elif i == 2:
                        emit_proj_qk(1)
                    elif i == 3:
                        emit_proj_v_half(1, 0)
                elif s == 1:
                    if i == 0:
                        emit_proj_v_half(1, 1)
                    elif i == 5:
                        emit_proj_qk(2)
                    elif i == 6:
                        emit_proj_v_half(2, 0)
                    elif i == 7:
                        emit_proj_v_half(2, 1)
                elif s + 1 < NS:
                    if i == 5:
                        emit_proj_qk(s + 1)
                    elif i == 6:
                        emit_proj_v_half(s + 1, 0)
                    elif i == 7:
                        emit_proj_v_half(s + 1, 1)
                if oproj_work and g >= oproj_work[0][4]:
                    emit_oproj_qtile(oproj_work.pop(0))

            emit_proj_qk(0)
            emit_scores(stream[0])
            zaug = None
            prev_zaug = None
            for g in range(G):
                s, kb = stream[g][0], stream[g][1]
                emit_exp(stream[g])
                if g + 1 < G:
                    emit_scores(stream[g + 1])
                if g >= 1:
                    ps_, pkb = stream[g - 1][0], stream[g - 1][1]
                    if pkb == 0:
                        # stream[g-1] opens slice ps_: finish the previous
                        # slice's accumulators, then claim fresh ones
                        if prev_zaug is not None:
                            emit_zfinish(ps_ - 1, prev_zaug, g0=g)
                        zaug = [
                            zps.tile([HD + 1, SW], F32, tag="za", name="zauga"),
                            zps.tile([HD + 1, SW], F32, tag="zb", name="zaugb"),
                        ]
                        prev_zaug = zaug
                    emit_pv(stream[g - 1], zaug)
                inserts(s, g - first_of_slice[s], g)
            emit_pv(stream[G - 1], zaug)

            # tail: last slice's Z finish + remaining O-projections
            emit_zfinish(NS - 1, zaug, g0=G, last=True)
            while oproj_work:
                emit_oproj_qtile(oproj_work.pop(0), scalar_assist=True)

    _split_waits(nc)
    return nc


_NC_CACHE = {}


def _get_nc():
    if "nc" not in _NC_CACHE:
        _NC_CACHE["nc"] = build_nc()
    return _NC_CACHE["nc"]


def make_in_maps(combined_embed, W_K, b_K, W_Q, b_Q, W_V, b_V, W_O, b_O):
    f32 = np.float32
    in_maps = []
    for c in range(8):
        b = c // 4
        g = c % 4
        sl = slice(g * 128, (g + 1) * 128)
        xt = np.ascontiguousarray(np.asarray(combined_embed[b], f32).T)
        in_maps.append(
            {
                "xt": xt.astype(_BF16),
                "wq": np.ascontiguousarray(np.asarray(W_Q, f32)[:, sl]).astype(_BF16),
                "wk": np.ascontiguousarray(np.asarray(W_K, f32)[:, sl]).astype(_BF16),
                "wv": np.ascontiguousarray(np.asarray(W_V, f32)[:, sl]).astype(_BF16),
                "wo": np.ascontiguousarray(np.asarray(W_O, f32)[sl, :]).astype(_BF16),
                "bq": np.asarray(b_Q, f32)[sl].reshape(128, 1).copy(),
                "bk": np.asarray(b_K, f32)[sl].reshape(128, 1).copy(),
                "bv": np.asarray(b_V, f32)[sl].reshape(1, 128).copy(),
            }
        )
    return in_maps


def run_cores(in_maps, **kwargs):
    nc = _get_nc()
    return run_bass_kernel_spmd(nc, in_maps, core_ids=list(range(8)), **kwargs)


def kernel(
    combined_embed, W_K, b_K, W_Q, b_Q, W_V, b_V, W_O, b_O
):  # full inputs -> full output
    in_maps = make_in_maps(
        combined_embed, W_K, b_K, W_Q, b_Q, W_V, b_V, W_O, b_O
    )
    res = run_cores(in_maps)
    out = np.zeros((B, T, D), np.float32)
    for c in range(8):
        out[c // 4] += res.results[c]["out"]
    out += np.asarray(b_O, np.float32)[None, None, :]
    return out


# revision 17
# speedup vs baseline: 1.3117x; 1.0001x over previous
"""Multi-head causal self-attention on 8 TRN2 NeuronCores.

Problem: B=2, T=4096, D=512, H=8 heads (hd=64), fp32 in/out.

Sharding: core c in 0..7 handles batch b = c//4 and head pair g = c%4
(heads 2g, 2g+1 -> D-slice [128g, 128g+128)). Each core computes
    partial_out = concat_h( softmax(causal(Q_h K_h^T / 8)) V_h ) @ W_O[slice]
for its two heads; the host sums the 4 partials per batch and adds b_O.

Pipeline design (v4). ScalarE exp() is the hard floor (~17M causal score
elements per core at 1 elem/cycle/lane @1.2GHz) and TensorE is a close
second, so everything is organised to keep both streams dense and warm
(the HAM clock gate halves the PE clock after ~3.4us of idle):

  - Score slot = ONE key block x BOTH heads in one [128,1024] PSUM tile
    (head A bank 0, head B bank 1), one exp ACTIVATE per slot through a
    [128,2,n] strided AP -> 144 calls total.
  - One GLOBAL unit stream across all q-slices: scores run 1 unit ahead
    of exp and PV runs 1 unit behind, ACROSS slice boundaries, so the
    exp stream never waits at a boundary.
  - QKV projection (slice s+1) and O-projection slots live in their OWN
    2-bank PSUM pool ("ex"), so they never steal slots from the
    exp-paced score rotation (which would stall the exp stream by one
    exp period per stolen slot).
  - Normalisation happens AFTER the O-projection: each head's O-proj is
    a separate 64-contraction matmul (row-tiled, concurrent), and the
    PSUM->SBUF evac scales by 1/L per output row (per-partition scalar)
    and fuses the two heads' add:  o = ps_A * (1/L_A) + ps_B * (1/L_B).
    1/L reaches [128,1] partition form via one DRAM-bounce spread per
    head per slice; no broadcast round-trip, no Z normalisation pass.
  - V bias is a DVE broadcast-add fused into the V PSUM->SBUF evac; each
    V t-tile owns a full PSUM bank (PE-write + DVE-read of the same bank
    is a fatal collision).
  - exp table set is preloaded with a dummy call during the DMA phase;
    input DMAs are ordered so the slice-0 projection starts ASAP.
"""

import numpy as np

import concourse.bass as bass
import concourse.mybir as mybir
from concourse.tile import TileContext
from concourse.bass_utils import run_bass_kernel_spmd

try:
    import ml_dtypes

    _BF16 = ml_dtypes.bfloat16
except ImportError:  # pragma: no cover
    _BF16 = None

F32 = mybir.dt.float32
BF16 = mybir.dt.bfloat16

B, T, D, H = 2, 4096, 512, 8
HD = D // H  # 64
SW = 512  # q-slice width
NS = T // SW  # 8 q-slices
NKC = D // 128  # 4 contraction chunks for the projections
NTT = T // 128  # 32 t-tiles / key blocks
NEG = -1.0e9


def _split_waits(nc, max_waits=1):
    """The staged walrus rejects >1 semaphore wait per instruction; hoist
    extras onto same-engine NoOps inserted right before the instruction."""
    counter = 0
    for f in nc.m.functions:
        for blk in f.blocks:
            insts = blk.instructions
            out, changed = [], False
            for ins in insts:
                si = getattr(ins, "sync_info", None)
                waits = list(si.on_wait) if si is not None and si.on_wait else []
                if len(waits) > max_waits:
                    changed = True
                    for w in waits[:-max_waits]:
                        counter += 1
                        nop = mybir.InstNoOp(
                            name=f"I-wsplit-{counter}",
                            engine=ins.engine,
                            ins=[],
                            outs=[],
                        )
                        nop.sync_info = mybir.SyncInfo(on_wait=[w], on_update=[])
                        out.append(nop)
                    ins.sync_info = mybir.SyncInfo(
                        on_wait=waits[-max_waits:], on_update=list(si.on_update)
                    )
                out.append(ins)
            if changed:
                blk.instructions = out
    return counter


def build_nc():
    nc = bass.Bass("TRN2")

    xt = nc.dram_tensor("xt", [D, T], BF16, kind="ExternalInput")
    wq = nc.dram_tensor("wq", [D, 128], BF16, kind="ExternalInput")
    wk = nc.dram_tensor("wk", [D, 128], BF16, kind="ExternalInput")
    wv = nc.dram_tensor("wv", [D, 128], BF16, kind="ExternalInput")
    wo = nc.dram_tensor("wo", [128, D], BF16, kind="ExternalInput")
    bq = nc.dram_tensor("bq", [128, 1], F32, kind="ExternalInput")
    bk = nc.dram_tensor("bk", [128, 1], F32, kind="ExternalInput")
    bv = nc.dram_tensor("bv", [1, 128], F32, kind="ExternalInput")
    out = nc.dram_tensor("out", [T, D], F32, kind="ExternalOutput")

    # maskneg[k, q'] = 0 where q' >= k else NEG  (S^T diagonal subtile mask)
    ident_np = np.eye(128, dtype=np.float32)
    mask_np = np.where(
        np.arange(128)[None, :] >= np.arange(128)[:, None], 0.0, NEG
    ).astype(np.float32)
    ident_dram = nc.inline_tensor(ident_np.astype(_BF16), name="identc")
    mask_dram = nc.inline_tensor(mask_np.astype(_BF16), name="maskc")

    with TileContext(nc) as tc:
        with (
            tc.tile_pool(name="singles", bufs=1) as singles,
            tc.tile_pool(name="ps", bufs=2, space="PSUM") as ps,
            tc.tile_pool(name="ex", bufs=1, space="PSUM") as ext,
            tc.tile_pool(name="zps", bufs=1, space="PSUM") as zps,
            tc.tile_pool(name="pt", bufs=6) as ptp,
            tc.tile_pool(name="sl", bufs=2) as slp,
            tc.tile_pool(name="zn", bufs=3) as znp,
            tc.tile_pool(name="outp", bufs=4) as outp,
            tc.tile_pool(name="drp", bufs=2, space="DRAM") as drp,
        ):
            # ---- static SBUF; DMA order matters: slice-0 projection inputs
            # first (each weight is ONE batched DMA -- the sync queue costs
            # ~600ns of issue time per descriptor), bulk xt via the idle
            # GpSimd SWDGE queue ----
            xt_sb = [
                [
                    singles.tile(
                        [128, SW], BF16, tag=f"xt{c}_{s}", name=f"xt_sb{c}_{s}"
                    )
                    for s in range(NS)
                ]
                for c in range(NKC)
            ]
            for c in range(NKC):
                nc.sync.dma_start(
                    out=xt_sb[c][0][:, :], in_=xt[c * 128 : (c + 1) * 128, 0:SW]
                )
            wq_sb = singles.tile([128, NKC, 128], BF16, tag="wq")
            wk_sb = singles.tile([128, NKC, 128], BF16, tag="wk")
            wv_sb = singles.tile([128, NKC, 128], BF16, tag="wv")
            nc.sync.dma_start(
                out=wq_sb[:, :, :], in_=wq[:, :].rearrange("(c p) n -> p c n", p=128)
            )
            bq_sb = singles.tile([128, 1], F32, tag="bq")
            nc.sync.dma_start(out=bq_sb[:, :], in_=bq[:, :])
            # preload the exp table set while the rest of the DMAs stream in
            warm_sb = singles.tile([1, 1], BF16, tag="warm")
            nc.scalar.activation(
                out=warm_sb[:, :],
                in_=bq_sb[0:1, 0:1],
                func=mybir.ActivationFunctionType.Exp,
                scale=0.125,
            )
            ident_sb = singles.tile([128, 128], BF16, tag="ident")
            mask_sb = singles.tile([128, 128], BF16, tag="mask")
            nc.sync.dma_start(out=ident_sb[:, :], in_=ident_dram[:, :])
            nc.sync.dma_start(out=mask_sb[:, :], in_=mask_dram[:, :])
            bk_sb = singles.tile([128, 1], F32, tag="bk")
            nc.sync.dma_start(out=bk_sb[:, :], in_=bk[:, :])
            nc.sync.dma_start(
                out=wk_sb[:, :, :], in_=wk[:, :].rearrange("(c p) n -> p c n", p=128)
            )
            nc.sync.dma_start(
                out=wv_sb[:, :, :], in_=wv[:, :].rearrange("(c p) n -> p c n", p=128)
            )
            # b_V broadcast across all 128 partitions: [128, 128] f32
            bvrep_sb = singles.tile([128, 128], F32, tag="bvrep")
            bvap = bv[:, :]
            bv_src = bass.AP(
                tensor=bvap.tensor,
                offset=bvap.offset,
                ap=[[0, 128]] + list(bvap.ap[1:]),
            )
            nc.sync.dma_start(out=bvrep_sb[:, :], in_=bv_src)
            wo_sb = singles.tile([128, D], BF16, tag="wo")
            nc.sync.dma_start(out=wo_sb[:, :], in_=wo[:, :])
            for s in range(1, NS):
                for c in range(NKC):
                    nc.gpsimd.dma_start(
                        out=xt_sb[c][s][:, :],
                        in_=xt[c * 128 : (c + 1) * 128, s * SW : (s + 1) * SW],
                    )

            qt_sb = [
                singles.tile([128, SW], BF16, tag=f"qt{s}", name=f"qt_sb{s}")
                for s in range(NS)
            ]
            kt_sb = [
                singles.tile([128, SW], BF16, tag=f"kt{s}", name=f"kt_sb{s}")
                for s in range(NS)
            ]
            # V_aug pair per key block: [128(t), 130]; cols 0:64 head A,
            # col 64 ones(A), cols 65:129 head B, col 129 ones(B)
            va_sb = [
                singles.tile([128, 2 * (HD + 1)], BF16, tag=f"va{t}", name=f"va_sb{t}")
                for t in range(NTT)
            ]
            for t in range(NTT):
                nc.vector.memset(va_sb[t][:, HD : HD + 1], 1.0)
                nc.vector.memset(va_sb[t][:, 2 * HD + 1 : 2 * HD + 2], 1.0)

            hrows = (slice(0, HD), slice(HD, 128))

            # ---------- emit helpers ----------
            def emit_proj_qk(s):
                sg = ext.tile([128, 2 * SW], F32, tag="ex", name="ps_qk")
                for idx, (w_sb, b_sb, dst) in enumerate(
                    ((wq_sb, bq_sb, qt_sb[s]), (wk_sb, bk_sb, kt_sb[s]))
                ):
                    off = idx * SW
                    for c in range(NKC):
                        nc.tensor.matmul(
                            sg[:, off : off + SW],
                            lhsT=w_sb[:, c, :],
                            rhs=xt_sb[c][s][:, :],
                            start=(c == 0),
                            stop=(c == NKC - 1),
                            skip_group_check=True,
                        )
                    nc.vector.tensor_scalar_add(
                        dst[:, :], sg[:, off : off + SW], b_sb[:, :]
                    )

            def emit_proj_v_half(s, half):
                # one full PSUM bank per t-tile: a DVE evac of one t-tile may
                # run while PE still writes another, and PE-write + DVE-read
                # in the SAME bank is a fatal collision
                if True:
                    sg = ext.tile([128, 2 * SW], F32, tag="ex", name="ps_v")
                    for tt in (2 * half, 2 * half + 1):
                        t = 4 * s + tt
                        off = (tt % 2) * SW
                        for c in range(NKC):
                            nc.tensor.matmul(
                                sg[:, off : off + 128],
                                lhsT=xt_sb[c][s][:, tt * 128 : (tt + 1) * 128],
                                rhs=wv_sb[:, c, :],
                                start=(c == 0),
                                stop=(c == NKC - 1),
                                skip_group_check=True,
                            )
                        # evac + b_V add in one op: dst [128,2,64] strided
                        # (skip the ones columns), src/bias [128,2,64]
                        dst3 = va_sb[t][:, 0 : 2 * (HD + 1)].rearrange(
                            "p (a b) -> p a b", a=2
                        )[:, :, 0:HD]
                        src3 = sg[:, off : off + 128].rearrange(
                            "p (a b) -> p a b", a=2
                        )
                        bv3 = bvrep_sb[:, :].rearrange("p (a b) -> p a b", a=2)
                        nc.vector.tensor_add(dst3, src3, bv3)

            def emit_scores(unit):
                s, kb, n, qlo = unit[:4]
                qs = s * SW
                diag = kb * 128 >= qs
                sg = ps.tile([128, 2 * SW], F32, tag="sg", name="ps_sg")
                unit[4] = sg
                for h in range(2):
                    off = h * SW
                    nc.tensor.matmul(
                        sg[:, off : off + n],
                        lhsT=kt_sb[kb // 4][
                            hrows[h], (kb % 4) * 128 : (kb % 4 + 1) * 128
                        ],
                        rhs=qt_sb[s][hrows[h], qlo - qs : qlo - qs + n],
                        start=True,
                        stop=not diag,
                        skip_group_check=True,
                        tile_position=(h * HD, 0),
                    )
                if diag:
                    for h in range(2):
                        nc.tensor.matmul(
                            sg[:, h * SW : h * SW + 128],
                            lhsT=ident_sb[:, :],
                            rhs=mask_sb[:, :],
                            start=False,
                            stop=True,
                            skip_group_check=True,
                        )

            def emit_exp(unit):
                s, kb, n, qlo, sg = unit[:5]
                pt = ptp.tile([128, 2 * SW], BF16, tag="pt", name="pt")
                in3 = sg[:, :].rearrange("p (a b) -> p a b", a=2)[:, :, 0:n]
                out3 = pt[:, 0 : 2 * n].rearrange("p (a b) -> p a b", a=2)
                nc.scalar.activation(
                    out=out3,
                    in_=in3,
                    func=mybir.ActivationFunctionType.Exp,
                    scale=0.125,
                )
                unit[4] = pt

            def emit_pv(unit, zaug):
                s, kb, n, qlo, pt = unit[:5]
                qs = s * SW
                nkb = 4 * (s + 1)
                for h in range(2):
                    nc.tensor.matmul(
                        zaug[h][0 : HD + 1, qlo - qs : SW],
                        lhsT=va_sb[kb][:, h * (HD + 1) : (h + 1) * (HD + 1)],
                        rhs=pt[:, h * n : h * n + n],
                        start=(kb == 0),
                        stop=(kb == nkb - 1),
                        skip_group_check=True,
                    )

            # O-proj work items: (zpair, li_pair_or_ri_pair, qs, j, ready_at).
            # Drained one q-tile per stream iteration once ready_at passes,
            # so the 1.3us of DVE evac per q-tile never bunches up.  The
            # reciprocals are deferred to the first q-tile item (j==0): in
            # emit_zfinish they would block the DVE FIFO on the L-spread
            # DRAM round-trip.
            oproj_work = []

            def emit_zfinish(sp, zaug, g0=0, last=False):
                """Evacuate slice sp's Z/L accumulators and set up the
                deferred, post-O-proj normalisation: unnormalised Z pair
                stacked [128,512] bf16 (head B shifted to partitions
                64..127 via GpSimd) + per-q-tile [128,1] 1/L spreads."""
                zpair = znp.tile([128, SW], BF16, tag="zn")
                znb = slp.tile([HD, SW], BF16, tag="znb")
                nc.vector.tensor_copy(zpair[0:HD, :], zaug[0][0:HD, :])
                nc.vector.tensor_copy(znb[:, :], zaug[1][0:HD, :])
                lri = []
                for h in range(2):
                    lr = slp.tile([1, SW], F32, tag=f"lr{h}", name="lrow")
                    if last:
                        # ScalarE is idle at the tail; take it off DVE
                        nc.scalar.copy(lr[:, :], zaug[h][HD : HD + 1, :])
                    else:
                        nc.vector.tensor_copy(lr[:, :], zaug[h][HD : HD + 1, :])
                    rd = drp.tile([1, SW], F32, tag=f"rd{h}", name="rd")
                    nc.gpsimd.dma_start(out=rd[:, :], in_=lr[:, :])
                    # [1,512] -> [128,4]: partition p, col j = L[j*128+p]
                    li = znp.tile([128, SW // 128], F32, tag=f"li{h}", name="li")
                    nc.gpsimd.dma_start(
                        out=li[:, :],
                        in_=rd[0, :].rearrange("(a p) -> p a", p=128),
                    )
                    lri.append(li)
                nc.gpsimd.dma_start(out=zpair[HD:128, :], in_=znb[:, :])
                state = {"li": lri, "ri": None}
                for j in range(4):
                    oproj_work.append((zpair, state, sp * SW, j, g0 + 8 + 2 * j))

            def emit_oproj_qtile(item, scalar_assist=False):
                zpair, state, qs_t, j, _ = item
                if state["ri"] is None:
                    ri_pair = []
                    for h in range(2):
                        ri = znp.tile(
                            [128, SW // 128], F32, tag=f"ri{h}", name="ri"
                        )
                        nc.vector.reciprocal(ri[:, :], state["li"][h][:, :])
                        ri_pair.append(ri)
                    state["ri"] = ri_pair
                ra, rb = state["ri"]
                op = ext.tile([128, 2 * SW], F32, tag="ex", name="ps_o")
                jq = slice(j * 128, (j + 1) * 128)
                nc.tensor.matmul(
                    op[:, 0:SW],
                    lhsT=zpair[0:HD, jq],
                    rhs=wo_sb[0:HD, :],
                    start=True,
                    stop=True,
                    skip_group_check=True,
                    tile_position=(0, 0),
                )
                nc.tensor.matmul(
                    op[:, SW : 2 * SW],
                    lhsT=zpair[HD:128, jq],
                    rhs=wo_sb[HD:128, :],
                    start=True,
                    stop=True,
                    skip_group_check=True,
                    tile_position=(HD, 0),
                )
                o_tmp = outp.tile([128, D], F32, tag="otmp", name="o_tmp")
                if scalar_assist:
                    nc.scalar.mul(o_tmp[:, :], op[:, SW : 2 * SW], rb[:, j : j + 1])
                else:
                    nc.vector.tensor_scalar_mul(
                        o_tmp[:, :], op[:, SW : 2 * SW], rb[:, j : j + 1]
                    )
                o_sb = outp.tile([128, D], F32, tag="ot", name="o_sb")
                nc.vector.scalar_tensor_tensor(
                    o_sb[:, :],
                    op[:, 0:SW],
                    ra[:, j : j + 1],
                    o_tmp[:, :],
                    op0=mybir.AluOpType.mult,
                    op1=mybir.AluOpType.add,
                )
                r0 = qs_t + j * 128
                nc.sync.dma_start(out=out[r0 : r0 + 128, :], in_=o_sb[:, :])

            # HAM warm-up: the PE clock gate needs ~3.4us of sustained
            # activity to reach 2.4GHz; burn the DMA-wait on dummy matmuls
            # so slice 0 runs warm.
            warm_ps = ext.tile([128, 2 * SW], F32, tag="ex", name="ps_warm")
            for _ in range(16):
                nc.tensor.matmul(
                    warm_ps[:, 0:SW],
                    lhsT=ident_sb[:, :],
                    rhs=xt_sb[0][0][:, :],
                    start=True,
                    stop=True,
                    skip_group_check=True,
                )

            # ---------- global unit stream ----------
            stream = []
            first_of_slice = {}
            for s in range(NS):
                qs = s * SW
                first_of_slice[s] = len(stream)
                for kb in range(4 * (s + 1)):
                    qlo = max(qs, kb * 128)
                    stream.append([s, kb, qs + SW - qlo, qlo, None])
            G = len(stream)

            def inserts(s, i, g):
                # extras schedule (one PSUM "ex" slot each, at most ~1/iter):
                # proj for slice s+1 must be fully emitted before the scores
                # of s+1 u0 (iter L-1); V halves feed PV with plenty of lag
                if s == 0:
                    if i == 0:
                        emit_proj_v_half(0, 0)
                    elif i == 1:
                        emit_proj_v_half(0, 1)
                    elif i == 2:
                        emit_proj_qk(1)
                    elif i == 3:
                        emit_proj_v_half(1, 0)
                elif s == 1:
                    if i == 0:
                        emit_proj_v_half(1, 1)
                    elif i == 5:
                        emit_proj_qk(2)
                    elif i == 6:
                        emit_proj_v_half(2, 0)
                    elif i == 7:
                        emit_proj_v_half(2, 1)
                elif s + 1 < NS:
                    if i == 5:
                        emit_proj_qk(s + 1)
                    elif i == 6:
                        emit_proj_v_half(s + 1, 0)
                    elif i == 7:
                        emit_proj_v_half(s + 1, 1)
                if oproj_work and g >= oproj_work[0][4]:
                    emit_oproj_qtile(oproj_work.pop(0))

            emit_proj_qk(0)
            emit_scores(stream[0])
            zaug = None
            prev_zaug = None
            for g in range(G):
                s, kb = stream[g][0], stream[g][1]
                emit_exp(stream[g])
                if g + 1 < G:
                    emit_scores(stream[g + 1])
                if g >= 1:
                    ps_, pkb = stream[g - 1][0], stream[g - 1][1]
                    if pkb == 0:
                        # stream[g-1] opens slice ps_: finish the previous
                        # slice's accumulators, then claim fresh ones
                        if prev_zaug is not None:
                            emit_zfinish(ps_ - 1, prev_zaug, g0=g)
                        zaug = [
                            zps.tile([HD + 1, SW], F32, tag="za", name="zauga"),
                            zps.tile([HD + 1, SW], F32, tag="zb", name="zaugb"),
                        ]
                        prev_zaug = zaug
                    emit_pv(stream[g - 1], zaug)
                inserts(s, g - first_of_slice[s], g)
            emit_pv(stream[G - 1], zaug)

            # tail: last slice's Z finish + remaining O-projections
            emit_zfinish(NS - 1, zaug, g0=G, last=True)
            while oproj_work:
                emit_oproj_qtile(oproj_work.pop(0), scalar_assist=True)

    _split_waits(nc)
    return nc


_NC_CACHE = {}


def _get_nc():
    if "nc" not in _NC_CACHE:
        _NC_CACHE["nc"] = build_nc()
    return _NC_CACHE["nc"]


def make_in_maps(combined_embed, W_K, b_K, W_Q, b_Q, W_V, b_V, W_O, b_O):
    f32 = np.float32
    in_maps = []
    for c in range(8):
        b = c // 4
        g = c % 4
        sl = slice(g * 128, (g + 1) * 128)
        xt = np.ascontiguousarray(np.asarray(combined_embed[b], f32).T)
        in_maps.append(
            {
                "xt": xt.astype(_BF16),
                "wq": np.ascontiguousarray(np.asarray(W_Q, f32)[:, sl]).astype(_BF16),
                "wk": np.ascontiguousarray(np.asarray(W_K, f32)[:, sl]).astype(_BF16),
                "wv": np.ascontiguousarray(np.asarray(W_V, f32)[:, sl]).astype(_BF16),
                "wo": np.ascontiguousarray(np.asarray(W_O, f32)[sl, :]).astype(_BF16),
                "bq": np.asarray(b_Q, f32)[sl].reshape(128, 1).copy(),
                "bk": np.asarray(b_K, f32)[sl].reshape(128, 1).copy(),
                "bv": np.asarray(b_V, f32)[sl].reshape(1, 128).copy(),
            }
        )
    return in_maps


def run_cores(in_maps, **kwargs):
    nc = _get_nc()
    return run_bass_kernel_spmd(nc, in_maps, core_ids=list(range(8)), **kwargs)


def kernel(
    combined_embed, W_K, b_K, W_Q, b_Q, W_V, b_V, W_O, b_O
):  # full inputs -> full output
    in_maps = make_in_maps(
        combined_embed, W_K, b_K, W_Q, b_Q, W_V, b_V, W_O, b_O
    )
    res = run_cores(in_maps)
    out = np.zeros((B, T, D), np.float32)
    for c in range(8):
        out[c // 4] += res.results[c]["out"]
    out += np.asarray(b_O, np.float32)[None, None, :]
    return out


# revision 18
# speedup vs baseline: 1.3154x; 1.0029x over previous
"""Multi-head causal self-attention on 8 TRN2 NeuronCores.

Problem: B=2, T=4096, D=512, H=8 heads (hd=64), fp32 in/out.

Sharding: core c in 0..7 handles batch b = c//4 and head pair g = c%4
(heads 2g, 2g+1 -> D-slice [128g, 128g+128)). Each core computes
    partial_out = concat_h( softmax(causal(Q_h K_h^T / 8)) V_h ) @ W_O[slice]
for its two heads; the host sums the 4 partials per batch and adds b_O.

Pipeline design (v4). ScalarE exp() is the hard floor (~17M causal score
elements per core at 1 elem/cycle/lane @1.2GHz) and TensorE is a close
second, so everything is organised to keep both streams dense and warm
(the HAM clock gate halves the PE clock after ~3.4us of idle):

  - Score slot = ONE key block x BOTH heads in one [128,1024] PSUM tile
    (head A bank 0, head B bank 1), one exp ACTIVATE per slot through a
    [128,2,n] strided AP -> 144 calls total.
  - One GLOBAL unit stream across all q-slices: scores run 1 unit ahead
    of exp and PV runs 1 unit behind, ACROSS slice boundaries, so the
    exp stream never waits at a boundary.
  - QKV projection (slice s+1) and O-projection slots live in their OWN
    2-bank PSUM pool ("ex"), so they never steal slots from the
    exp-paced score rotation (which would stall the exp stream by one
    exp period per stolen slot).
  - Normalisation happens AFTER the O-projection: each head's O-proj is
    a separate 64-contraction matmul (row-tiled, concurrent), and the
    PSUM->SBUF evac scales by 1/L per output row (per-partition scalar)
    and fuses the two heads' add:  o = ps_A * (1/L_A) + ps_B * (1/L_B).
    1/L reaches [128,1] partition form via one DRAM-bounce spread per
    head per slice; no broadcast round-trip, no Z normalisation pass.
  - V bias is a DVE broadcast-add fused into the V PSUM->SBUF evac; each
    V t-tile owns a full PSUM bank (PE-write + DVE-read of the same bank
    is a fatal collision).
  - exp table set is preloaded with a dummy call during the DMA phase;
    input DMAs are ordered so the slice-0 projection starts ASAP.
"""

import numpy as np

import concourse.bass as bass
import concourse.mybir as mybir
from concourse.tile import TileContext
from concourse.bass_utils import run_bass_kernel_spmd

try:
    import ml_dtypes

    _BF16 = ml_dtypes.bfloat16
except ImportError:  # pragma: no cover
    _BF16 = None

F32 = mybir.dt.float32
BF16 = mybir.dt.bfloat16

B, T, D, H = 2, 4096, 512, 8
HD = D // H  # 64
SW = 512  # q-slice width
NS = T // SW  # 8 q-slices
NKC = D // 128  # 4 contraction chunks for the projections
NTT = T // 128  # 32 t-tiles / key blocks
NEG = -1.0e9


def _split_waits(nc, max_waits=1):
    """The staged walrus rejects >1 semaphore wait per instruction; hoist
    extras onto same-engine NoOps inserted right before the instruction."""
    counter = 0
    for f in nc.m.functions:
        for blk in f.blocks:
            insts = blk.instructions
            out, changed = [], False
            for ins in insts:
                si = getattr(ins, "sync_info", None)
                waits = list(si.on_wait) if si is not None and si.on_wait else []
                if len(waits) > max_waits:
                    changed = True
                    for w in waits[:-max_waits]:
                        counter += 1
                        nop = mybir.InstNoOp(
                            name=f"I-wsplit-{counter}",
                            engine=ins.engine,
                            ins=[],
                            outs=[],
                        )
                        nop.sync_info = mybir.SyncInfo(on_wait=[w], on_update=[])
                        out.append(nop)
                    ins.sync_info = mybir.SyncInfo(
                        on_wait=waits[-max_waits:], on_update=list(si.on_update)
                    )
                out.append(ins)
            if changed:
                blk.instructions = out
    return counter


def build_nc():
    nc = bass.Bass("TRN2")

    xt = nc.dram_tensor("xt", [D, T], BF16, kind="ExternalInput")
    wq = nc.dram_tensor("wq", [D, 128], BF16, kind="ExternalInput")
    wk = nc.dram_tensor("wk", [D, 128], BF16, kind="ExternalInput")
    wv = nc.dram_tensor("wv", [D, 128], BF16, kind="ExternalInput")
    wo = nc.dram_tensor("wo", [128, D], BF16, kind="ExternalInput")
    bq = nc.dram_tensor("bq", [128, 1], F32, kind="ExternalInput")
    bk = nc.dram_tensor("bk", [128, 1], F32, kind="ExternalInput")
    bv = nc.dram_tensor("bv", [1, 128], F32, kind="ExternalInput")
    out = nc.dram_tensor("out", [T, D], F32, kind="ExternalOutput")

    # maskneg[k, q'] = 0 where q' >= k else NEG  (S^T diagonal subtile mask)
    ident_np = np.eye(128, dtype=np.float32)
    mask_np = np.where(
        np.arange(128)[None, :] >= np.arange(128)[:, None], 0.0, NEG
    ).astype(np.float32)
    ident_dram = nc.inline_tensor(ident_np.astype(_BF16), name="identc")
    mask_dram = nc.inline_tensor(mask_np.astype(_BF16), name="maskc")

    with TileContext(nc) as tc:
        with (
            tc.tile_pool(name="singles", bufs=1) as singles,
            tc.tile_pool(name="ps", bufs=2, space="PSUM") as ps,
            tc.tile_pool(name="ex", bufs=1, space="PSUM") as ext,
            tc.tile_pool(name="zps", bufs=1, space="PSUM") as zps,
            tc.tile_pool(name="pt", bufs=6) as ptp,
            tc.tile_pool(name="sl", bufs=2) as slp,
            tc.tile_pool(name="zn", bufs=3) as znp,
            tc.tile_pool(name="outp", bufs=4) as outp,
            tc.tile_pool(name="drp", bufs=2, space="DRAM") as drp,
        ):
            # ---- static SBUF; DMA order matters: slice-0 projection inputs
            # first (each weight is ONE batched DMA -- the sync queue costs
            # ~600ns of issue time per descriptor), bulk xt via the idle
            # GpSimd SWDGE queue ----
            ident_sb = singles.tile([128, 128], BF16, tag="ident")
            mask_sb = singles.tile([128, 128], BF16, tag="mask")
            nc.sync.dma_start(out=ident_sb[:, :], in_=ident_dram[:, :])
            nc.sync.dma_start(out=mask_sb[:, :], in_=mask_dram[:, :])
            xt_sb = [
                [
                    singles.tile(
                        [128, SW], BF16, tag=f"xt{c}_{s}", name=f"xt_sb{c}_{s}"
                    )
                    for s in range(NS)
                ]
                for c in range(NKC)
            ]
            for c in range(NKC):
                nc.sync.dma_start(
                    out=xt_sb[c][0][:, :], in_=xt[c * 128 : (c + 1) * 128, 0:SW]
                )
            wq_sb = singles.tile([128, NKC, 128], BF16, tag="wq")
            wk_sb = singles.tile([128, NKC, 128], BF16, tag="wk")
            wv_sb = singles.tile([128, NKC, 128], BF16, tag="wv")
            nc.sync.dma_start(
                out=wq_sb[:, :, :], in_=wq[:, :].rearrange("(c p) n -> p c n", p=128)
            )
            bq_sb = singles.tile([128, 1], F32, tag="bq")
            nc.sync.dma_start(out=bq_sb[:, :], in_=bq[:, :])
            # preload the exp table set while the rest of the DMAs stream in
            warm_sb = singles.tile([1, 1], BF16, tag="warm")
            nc.scalar.activation(
                out=warm_sb[:, :],
                in_=bq_sb[0:1, 0:1],
                func=mybir.ActivationFunctionType.Exp,
                scale=0.125,
            )
            bk_sb = singles.tile([128, 1], F32, tag="bk")
            nc.sync.dma_start(out=bk_sb[:, :], in_=bk[:, :])
            nc.sync.dma_start(
                out=wk_sb[:, :, :], in_=wk[:, :].rearrange("(c p) n -> p c n", p=128)
            )
            nc.sync.dma_start(
                out=wv_sb[:, :, :], in_=wv[:, :].rearrange("(c p) n -> p c n", p=128)
            )
            # b_V broadcast across all 128 partitions: [128, 128] f32
            bvrep_sb = singles.tile([128, 128], F32, tag="bvrep")
            bvap = bv[:, :]
            bv_src = bass.AP(
                tensor=bvap.tensor,
                offset=bvap.offset,
                ap=[[0, 128]] + list(bvap.ap[1:]),
            )
            nc.sync.dma_start(out=bvrep_sb[:, :], in_=bv_src)
            wo_sb = singles.tile([128, D], BF16, tag="wo")
            nc.sync.dma_start(out=wo_sb[:, :], in_=wo[:, :])
            for s in range(1, NS):
                for c in range(NKC):
                    nc.gpsimd.dma_start(
                        out=xt_sb[c][s][:, :],
                        in_=xt[c * 128 : (c + 1) * 128, s * SW : (s + 1) * SW],
                    )

            qt_sb = [
                singles.tile([128, SW], BF16, tag=f"qt{s}", name=f"qt_sb{s}")
                for s in range(NS)
            ]
            kt_sb = [
                singles.tile([128, SW], BF16, tag=f"kt{s}", name=f"kt_sb{s}")
                for s in range(NS)
            ]
            # V_aug pair per key block: [128(t), 130]; cols 0:64 head A,
            # col 64 ones(A), cols 65:129 head B, col 129 ones(B)
            va_sb = [
                singles.tile([128, 2 * (HD + 1)], BF16, tag=f"va{t}", name=f"va_sb{t}")
                for t in range(NTT)
            ]
            for t in range(NTT):
                nc.vector.memset(va_sb[t][:, HD : HD + 1], 1.0)
                nc.vector.memset(va_sb[t][:, 2 * HD + 1 : 2 * HD + 2], 1.0)

            hrows = (slice(0, HD), slice(HD, 128))

            # ---------- emit helpers ----------
            def emit_proj_qk(s):
                sg = ext.tile([128, 2 * SW], F32, tag="ex", name="ps_qk")
                for idx, (w_sb, b_sb, dst) in enumerate(
                    ((wq_sb, bq_sb, qt_sb[s]), (wk_sb, bk_sb, kt_sb[s]))
                ):
                    off = idx * SW
                    for c in range(NKC):
                        nc.tensor.matmul(
                            sg[:, off : off + SW],
                            lhsT=w_sb[:, c, :],
                            rhs=xt_sb[c][s][:, :],
                            start=(c == 0),
                            stop=(c == NKC - 1),
                            skip_group_check=True,
                        )
                    nc.vector.tensor_scalar_add(
                        dst[:, :], sg[:, off : off + SW], b_sb[:, :]
                    )

            def emit_proj_v_half(s, half):
                # one full PSUM bank per t-tile: a DVE evac of one t-tile may
                # run while PE still writes another, and PE-write + DVE-read
                # in the SAME bank is a fatal collision
                if True:
                    sg = ext.tile([128, 2 * SW], F32, tag="ex", name="ps_v")
                    for tt in (2 * half, 2 * half + 1):
                        t = 4 * s + tt
                        off = (tt % 2) * SW
                        for c in range(NKC):
                            nc.tensor.matmul(
                                sg[:, off : off + 128],
                                lhsT=xt_sb[c][s][:, tt * 128 : (tt + 1) * 128],
                                rhs=wv_sb[:, c, :],
                                start=(c == 0),
                                stop=(c == NKC - 1),
                                skip_group_check=True,
                            )
                        # evac + b_V add in one op: dst [128,2,64] strided
                        # (skip the ones columns), src/bias [128,2,64]
                        dst3 = va_sb[t][:, 0 : 2 * (HD + 1)].rearrange(
                            "p (a b) -> p a b", a=2
                        )[:, :, 0:HD]
                        src3 = sg[:, off : off + 128].rearrange(
                            "p (a b) -> p a b", a=2
                        )
                        bv3 = bvrep_sb[:, :].rearrange("p (a b) -> p a b", a=2)
                        nc.vector.tensor_add(dst3, src3, bv3)

            def emit_scores(unit):
                s, kb, n, qlo = unit[:4]
                qs = s * SW
                diag = kb * 128 >= qs
                sg = ps.tile([128, 2 * SW], F32, tag="sg", name="ps_sg")
                unit[4] = sg
                for h in range(2):
                    off = h * SW
                    nc.tensor.matmul(
                        sg[:, off : off + n],
                        lhsT=kt_sb[kb // 4][
                            hrows[h], (kb % 4) * 128 : (kb % 4 + 1) * 128
                        ],
                        rhs=qt_sb[s][hrows[h], qlo - qs : qlo - qs + n],
                        start=True,
                        stop=not diag,
                        skip_group_check=True,
                        tile_position=(h * HD, 0),
                    )
                if diag:
                    for h in range(2):
                        nc.tensor.matmul(
                            sg[:, h * SW : h * SW + 128],
                            lhsT=ident_sb[:, :],
                            rhs=mask_sb[:, :],
                            start=False,
                            stop=True,
                            skip_group_check=True,
                        )

            def emit_exp(unit):
                s, kb, n, qlo, sg = unit[:5]
                pt = ptp.tile([128, 2 * SW], BF16, tag="pt", name="pt")
                in3 = sg[:, :].rearrange("p (a b) -> p a b", a=2)[:, :, 0:n]
                out3 = pt[:, 0 : 2 * n].rearrange("p (a b) -> p a b", a=2)
                nc.scalar.activation(
                    out=out3,
                    in_=in3,
                    func=mybir.ActivationFunctionType.Exp,
                    scale=0.125,
                )
                unit[4] = pt

            def emit_pv(unit, zaug):
                s, kb, n, qlo, pt = unit[:5]
                qs = s * SW
                nkb = 4 * (s + 1)
                for h in range(2):
                    nc.tensor.matmul(
                        zaug[h][0 : HD + 1, qlo - qs : SW],
                        lhsT=va_sb[kb][:, h * (HD + 1) : (h + 1) * (HD + 1)],
                        rhs=pt[:, h * n : h * n + n],
                        start=(kb == 0),
                        stop=(kb == nkb - 1),
                        skip_group_check=True,
                    )

            # O-proj work items: (zpair, li_pair_or_ri_pair, qs, j, ready_at).
            # Drained one q-tile per stream iteration once ready_at passes,
            # so the 1.3us of DVE evac per q-tile never bunches up.  The
            # reciprocals are deferred to the first q-tile item (j==0): in
            # emit_zfinish they would block the DVE FIFO on the L-spread
            # DRAM round-trip.
            oproj_work = []

            def emit_zfinish(sp, zaug, g0=0, last=False):
                """Evacuate slice sp's Z/L accumulators and set up the
                deferred, post-O-proj normalisation: unnormalised Z pair
                stacked [128,512] bf16 (head B shifted to partitions
                64..127 via GpSimd) + per-q-tile [128,1] 1/L spreads."""
                zpair = znp.tile([128, SW], BF16, tag="zn")
                znb = slp.tile([HD, SW], BF16, tag="znb")
                nc.vector.tensor_copy(zpair[0:HD, :], zaug[0][0:HD, :])
                nc.vector.tensor_copy(znb[:, :], zaug[1][0:HD, :])
                lri = []
                for h in range(2):
                    lr = slp.tile([1, SW], F32, tag=f"lr{h}", name="lrow")
                    if last:
                        # ScalarE is idle at the tail; take it off DVE
                        nc.scalar.copy(lr[:, :], zaug[h][HD : HD + 1, :])
                    else:
                        nc.vector.tensor_copy(lr[:, :], zaug[h][HD : HD + 1, :])
                    eng = nc.sync if h == 0 else nc.gpsimd
                    rd = drp.tile([1, SW], F32, tag=f"rd{h}", name="rd")
                    eng.dma_start(out=rd[:, :], in_=lr[:, :])
                    # [1,512] -> [128,4]: partition p, col j = L[j*128+p]
                    li = znp.tile([128, SW // 128], F32, tag=f"li{h}", name="li")
                    eng.dma_start(
                        out=li[:, :],
                        in_=rd[0, :].rearrange("(a p) -> p a", p=128),
                    )
                    lri.append(li)
                nc.gpsimd.dma_start(out=zpair[HD:128, :], in_=znb[:, :])
                state = {"li": lri, "ri": None}
                for j in range(4):
                    oproj_work.append((zpair, state, sp * SW, j, g0 + 10 + 2 * j))

            def emit_oproj_qtile(item, scalar_assist=False):
                zpair, state, qs_t, j, _ = item
                if state["ri"] is None:
                    ri_pair = []
                    for h in range(2):
                        ri = znp.tile(
                            [128, SW // 128], F32, tag=f"ri{h}", name="ri"
                        )
                        nc.vector.reciprocal(ri[:, :], state["li"][h][:, :])
                        ri_pair.append(ri)
                    state["ri"] = ri_pair
                ra, rb = state["ri"]
                op = ext.tile([128, 2 * SW], F32, tag="ex", name="ps_o")
                jq = slice(j * 128, (j + 1) * 128)
                nc.tensor.matmul(
                    op[:, 0:SW],
                    lhsT=zpair[0:HD, jq],
                    rhs=wo_sb[0:HD, :],
                    start=True,
                    stop=True,
                    skip_group_check=True,
                    tile_position=(0, 0),
                )
                nc.tensor.matmul(
                    op[:, SW : 2 * SW],
                    lhsT=zpair[HD:128, jq],
                    rhs=wo_sb[HD:128, :],
                    start=True,
                    stop=True,
                    skip_group_check=True,
                    tile_position=(HD, 0),
                )
                o_tmp = outp.tile([128, D], F32, tag="otmp", name="o_tmp")
                if scalar_assist:
                    nc.scalar.mul(o_tmp[:, :], op[:, SW : 2 * SW], rb[:, j : j + 1])
                else:
                    nc.vector.tensor_scalar_mul(
                        o_tmp[:, :], op[:, SW : 2 * SW], rb[:, j : j + 1]
                    )
                o_sb = outp.tile([128, D], F32, tag="ot", name="o_sb")
                nc.vector.scalar_tensor_tensor(
                    o_sb[:, :],
                    op[:, 0:SW],
                    ra[:, j : j + 1],
                    o_tmp[:, :],
                    op0=mybir.AluOpType.mult,
                    op1=mybir.AluOpType.add,
                )
                r0 = qs_t + j * 128
                nc.sync.dma_start(out=out[r0 : r0 + 128, :], in_=o_sb[:, :])

            # HAM warm-up: the PE clock gate needs ~3.4us of sustained
            # activity to reach 2.4GHz; burn the DMA-wait on dummy matmuls
            # (dep only on the tiny ident/mask DMAs, which are issued first)
            # so slice 0 runs warm and the real work is never behind them.
            warm_ps = ext.tile([128, 2 * SW], F32, tag="ex", name="ps_warm")
            for _ in range(32):
                nc.tensor.matmul(
                    warm_ps[:, 0:128],
                    lhsT=ident_sb[:, :],
                    rhs=mask_sb[:, :],
                    start=True,
                    stop=True,
                    skip_group_check=True,
                )

            # ---------- global unit stream ----------
            stream = []
            first_of_slice = {}
            for s in range(NS):
                qs = s * SW
                first_of_slice[s] = len(stream)
                for kb in range(4 * (s + 1)):
                    qlo = max(qs, kb * 128)
                    stream.append([s, kb, qs + SW - qlo, qlo, None])
            G = len(stream)

            def inserts(s, i, g):
                # extras schedule (one PSUM "ex" slot each, at most ~1/iter):
                # proj for slice s+1 must be fully emitted before the scores
                # of s+1 u0 (iter L-1); V halves feed PV with plenty of lag
                if s == 0:
                    if i == 0:
                        emit_proj_v_half(0, 0)
                    elif i == 1:
                        emit_proj_v_half(0, 1)
                    elif i == 2:
                        emit_proj_qk(1)
                    elif i == 3:
                        emit_proj_v_half(1, 0)
                elif s == 1:
                    if i == 0:
                        emit_proj_v_half(1, 1)
                    elif i == 5:
                        emit_proj_qk(2)
                    elif i == 6:
                        emit_proj_v_half(2, 0)
                    elif i == 7:
                        emit_proj_v_half(2, 1)
                elif s + 1 < NS:
                    if i == 5:
                        emit_proj_qk(s + 1)
                    elif i == 6:
                        emit_proj_v_half(s + 1, 0)
                    elif i == 7:
                        emit_proj_v_half(s + 1, 1)
                if oproj_work and g >= oproj_work[0][4]:
                    emit_oproj_qtile(oproj_work.pop(0))

            emit_proj_qk(0)
            emit_scores(stream[0])
            zaug = None
            prev_zaug = None
            for g in range(G):
                s, kb = stream[g][0], stream[g][1]
                emit_exp(stream[g])
                if g + 1 < G:
                    emit_scores(stream[g + 1])
                if g >= 1:
                    ps_, pkb = stream[g - 1][0], stream[g - 1][1]
                    if pkb == 0:
                        # stream[g-1] opens slice ps_: finish the previous
                        # slice's accumulators, then claim fresh ones
                        if prev_zaug is not None:
                            emit_zfinish(ps_ - 1, prev_zaug, g0=g)
                        zaug = [
                            zps.tile([HD + 1, SW], F32, tag="za", name="zauga"),
                            zps.tile([HD + 1, SW], F32, tag="zb", name="zaugb"),
                        ]
                        prev_zaug = zaug
                    emit_pv(stream[g - 1], zaug)
                inserts(s, g - first_of_slice[s], g)
            emit_pv(stream[G - 1], zaug)

            # tail: last slice's Z finish + remaining O-projections
            emit_zfinish(NS - 1, zaug, g0=G, last=True)
            while oproj_work:
                emit_oproj_qtile(oproj_work.pop(0), scalar_assist=True)

    _split_waits(nc)
    return nc


_NC_CACHE = {}


def _get_nc():
    if "nc" not in _NC_CACHE:
        _NC_CACHE["nc"] = build_nc()
    return _NC_CACHE["nc"]


def make_in_maps(combined_embed, W_K, b_K, W_Q, b_Q, W_V, b_V, W_O, b_O):
    f32 = np.float32
    in_maps = []
    for c in range(8):
        b = c // 4
        g = c % 4
        sl = slice(g * 128, (g + 1) * 128)
        xt = np.ascontiguousarray(np.asarray(combined_embed[b], f32).T)
        in_maps.append(
            {
                "xt": xt.astype(_BF16),
                "wq": np.ascontiguousarray(np.asarray(W_Q, f32)[:, sl]).astype(_BF16),
                "wk": np.ascontiguousarray(np.asarray(W_K, f32)[:, sl]).astype(_BF16),
                "wv": np.ascontiguousarray(np.asarray(W_V, f32)[:, sl]).astype(_BF16),
                "wo": np.ascontiguousarray(np.asarray(W_O, f32)[sl, :]).astype(_BF16),
                "bq": np.asarray(b_Q, f32)[sl].reshape(128, 1).copy(),
                "bk": np.asarray(b_K, f32)[sl].reshape(128, 1).copy(),
                "bv": np.asarray(b_V, f32)[sl].reshape(1, 128).copy(),
            }
        )
    return in_maps


def run_cores(in_maps, **kwargs):
    nc = _get_nc()
    return run_bass_kernel_spmd(nc, in_maps, core_ids=list(range(8)), **kwargs)


def kernel(
    combined_embed, W_K, b_K, W_Q, b_Q, W_V, b_V, W_O, b_O
):  # full inputs -> full output
    in_maps = make_in_maps(
        combined_embed, W_K, b_K, W_Q, b_Q, W_V, b_V, W_O, b_O
    )
    res = run_cores(in_maps)
    out = np.zeros((B, T, D), np.float32)
    for c in range(8):
        out[c // 4] += res.results[c]["out"]
    out += np.asarray(b_O, np.float32)[None, None, :]
    return out


# revision 21
# speedup vs baseline: 1.3614x; 1.0350x over previous
"""Multi-head causal self-attention on 8 TRN2 NeuronCores.

Problem: B=2, T=4096, D=512, H=8 heads (hd=64), fp32 in/out.

Sharding: core c in 0..7 handles batch b = c//4 and head pair g = c%4
(heads 2g, 2g+1 -> D-slice [128g, 128g+128)). Each core computes
    partial_out = concat_h( softmax(causal(Q_h K_h^T / 8)) V_h ) @ W_O[slice]
for its two heads; the host sums the 4 partials per batch and adds b_O.

Pipeline design (v4). ScalarE exp() is the hard floor (~17M causal score
elements per core at 1 elem/cycle/lane @1.2GHz) and TensorE is a close
second, so everything is organised to keep both streams dense and warm
(the HAM clock gate halves the PE clock after ~3.4us of idle):

  - Score slot = ONE key block x BOTH heads in one [128,1024] PSUM tile
    (head A bank 0, head B bank 1), one exp ACTIVATE per slot through a
    [128,2,n] strided AP -> 144 calls total.
  - One GLOBAL unit stream across all q-slices: scores run 1 unit ahead
    of exp and PV runs 1 unit behind, ACROSS slice boundaries, so the
    exp stream never waits at a boundary.
  - QKV projection (slice s+1) and O-projection slots live in their OWN
    2-bank PSUM pool ("ex"), so they never steal slots from the
    exp-paced score rotation (which would stall the exp stream by one
    exp period per stolen slot).
  - Normalisation happens AFTER the O-projection: each head's O-proj is
    a separate 64-contraction matmul (row-tiled, concurrent), and the
    PSUM->SBUF evac scales by 1/L per output row (per-partition scalar)
    and fuses the two heads' add:  o = ps_A * (1/L_A) + ps_B * (1/L_B).
    1/L reaches [128,1] partition form via one DRAM-bounce spread per
    head per slice; no broadcast round-trip, no Z normalisation pass.
  - V bias is a DVE broadcast-add fused into the V PSUM->SBUF evac; each
    V t-tile owns a full PSUM bank (PE-write + DVE-read of the same bank
    is a fatal collision).
  - exp table set is preloaded with a dummy call during the DMA phase;
    input DMAs are ordered so the slice-0 projection starts ASAP.
"""

import numpy as np

import concourse.bass as bass
import concourse.mybir as mybir
from concourse.tile import TileContext
from concourse.bass_utils import run_bass_kernel_spmd

try:
    import ml_dtypes

    _BF16 = ml_dtypes.bfloat16
except ImportError:  # pragma: no cover
    _BF16 = None

F32 = mybir.dt.float32
BF16 = mybir.dt.bfloat16

B, T, D, H = 2, 4096, 512, 8
HD = D // H  # 64
SW = 512  # q-slice width
NS = T // SW  # 8 q-slices
NKC = D // 128  # 4 contraction chunks for the projections
NTT = T // 128  # 32 t-tiles / key blocks
NEG = -1.0e9


def _split_waits(nc, max_waits=1):
    """The staged walrus rejects >1 semaphore wait per instruction; hoist
    extras onto same-engine NoOps inserted right before the instruction."""
    counter = 0
    for f in nc.m.functions:
        for blk in f.blocks:
            insts = blk.instructions
            out, changed = [], False
            for ins in insts:
                si = getattr(ins, "sync_info", None)
                waits = list(si.on_wait) if si is not None and si.on_wait else []
                if len(waits) > max_waits:
                    changed = True
                    for w in waits[:-max_waits]:
                        counter += 1
                        nop = mybir.InstNoOp(
                            name=f"I-wsplit-{counter}",
                            engine=ins.engine,
                            ins=[],
                            outs=[],
                        )
                        nop.sync_info = mybir.SyncInfo(on_wait=[w], on_update=[])
                        out.append(nop)
                    ins.sync_info = mybir.SyncInfo(
                        on_wait=waits[-max_waits:], on_update=list(si.on_update)
                    )
                out.append(ins)
            if changed:
                blk.instructions = out
    return counter


def build_nc():
    nc = bass.Bass("TRN2")

    xt = nc.dram_tensor("xt", [D, T], BF16, kind="ExternalInput")
    wq = nc.dram_tensor("wq", [D, 128], BF16, kind="ExternalInput")
    wk = nc.dram_tensor("wk", [D, 128], BF16, kind="ExternalInput")
    wv = nc.dram_tensor("wv", [D, 128], BF16, kind="ExternalInput")
    wo = nc.dram_tensor("wo", [128, D], BF16, kind="ExternalInput")
    bq = nc.dram_tensor("bq", [128, 1], F32, kind="ExternalInput")
    bk = nc.dram_tensor("bk", [128, 1], F32, kind="ExternalInput")
    bv = nc.dram_tensor("bv", [1, 128], F32, kind="ExternalInput")
    out = nc.dram_tensor("out", [T, D], BF16, kind="ExternalOutput")

    # maskneg[k, q'] = 0 where q' >= k else NEG  (S^T diagonal subtile mask)
    ident_np = np.eye(128, dtype=np.float32)
    mask_np = np.where(
        np.arange(128)[None, :] >= np.arange(128)[:, None], 0.0, NEG
    ).astype(np.float32)
    ident_dram = nc.inline_tensor(ident_np.astype(_BF16), name="identc")
    mask_dram = nc.inline_tensor(mask_np.astype(_BF16), name="maskc")

    with TileContext(nc) as tc:
        with (
            tc.tile_pool(name="singles", bufs=1) as singles,
            tc.tile_pool(name="ps", bufs=2, space="PSUM") as ps,
            tc.tile_pool(name="ex", bufs=1, space="PSUM") as ext,
            tc.tile_pool(name="zps", bufs=1, space="PSUM") as zps,
            tc.tile_pool(name="pt", bufs=6) as ptp,
            tc.tile_pool(name="sl", bufs=2) as slp,
            tc.tile_pool(name="zn", bufs=3) as znp,
            tc.tile_pool(name="outp", bufs=4) as outp,
            tc.tile_pool(name="drp", bufs=2, space="DRAM") as drp,
        ):
            # ---- static SBUF; DMA order matters: slice-0 projection inputs
            # first (each weight is ONE batched DMA -- the sync queue costs
            # ~600ns of issue time per descriptor), bulk xt via the idle
            # GpSimd SWDGE queue ----
            ident_sb = singles.tile([128, 128], BF16, tag="ident")
            mask_sb = singles.tile([128, 128], BF16, tag="mask")
            nc.sync.dma_start(out=ident_sb[:, :], in_=ident_dram[:, :])
            nc.sync.dma_start(out=mask_sb[:, :], in_=mask_dram[:, :])
            xt_sb = [
                [
                    singles.tile(
                        [128, SW], BF16, tag=f"xt{c}_{s}", name=f"xt_sb{c}_{s}"
                    )
                    for s in range(NS)
                ]
                for c in range(NKC)
            ]
            for c in range(NKC):
                nc.sync.dma_start(
                    out=xt_sb[c][0][:, :], in_=xt[c * 128 : (c + 1) * 128, 0:SW]
                )
            wq_sb = singles.tile([128, NKC, 128], BF16, tag="wq")
            wk_sb = singles.tile([128, NKC, 128], BF16, tag="wk")
            wv_sb = singles.tile([128, NKC, 128], BF16, tag="wv")
            nc.sync.dma_start(
                out=wq_sb[:, :, :], in_=wq[:, :].rearrange("(c p) n -> p c n", p=128)
            )
            nc.sync.dma_start(
                out=wk_sb[:, :, :], in_=wk[:, :].rearrange("(c p) n -> p c n", p=128)
            )
            bq_sb = singles.tile([128, 1], F32, tag="bq")
            nc.sync.dma_start(out=bq_sb[:, :], in_=bq[:, :])
            # preload the exp table set while the rest of the DMAs stream in
            warm_sb = singles.tile([1, 1], BF16, tag="warm")
            nc.scalar.activation(
                out=warm_sb[:, :],
                in_=bq_sb[0:1, 0:1],
                func=mybir.ActivationFunctionType.Exp,
                scale=0.125,
            )
            bk_sb = singles.tile([128, 1], F32, tag="bk")
            nc.sync.dma_start(out=bk_sb[:, :], in_=bk[:, :])
            nc.sync.dma_start(
                out=wv_sb[:, :, :], in_=wv[:, :].rearrange("(c p) n -> p c n", p=128)
            )
            # b_V broadcast across all 128 partitions: [128, 128] f32
            bvrep_sb = singles.tile([128, 128], F32, tag="bvrep")
            bvap = bv[:, :]
            bv_src = bass.AP(
                tensor=bvap.tensor,
                offset=bvap.offset,
                ap=[[0, 128]] + list(bvap.ap[1:]),
            )
            nc.sync.dma_start(out=bvrep_sb[:, :], in_=bv_src)
            wo_sb = singles.tile([128, D], BF16, tag="wo")
            nc.sync.dma_start(out=wo_sb[:, :], in_=wo[:, :])
            for s in range(1, NS):
                for c in range(NKC):
                    nc.gpsimd.dma_start(
                        out=xt_sb[c][s][:, :],
                        in_=xt[c * 128 : (c + 1) * 128, s * SW : (s + 1) * SW],
                    )

            qt_sb = [
                singles.tile([128, SW], BF16, tag=f"qt{s}", name=f"qt_sb{s}")
                for s in range(NS)
            ]
            kt_sb = [
                singles.tile([128, SW], BF16, tag=f"kt{s}", name=f"kt_sb{s}")
                for s in range(NS)
            ]
            # V_aug pair per key block: [128(t), 130]; cols 0:64 head A,
            # col 64 ones(A), cols 65:129 head B, col 129 ones(B)
            va_sb = [
                singles.tile([128, 2 * (HD + 1)], BF16, tag=f"va{t}", name=f"va_sb{t}")
                for t in range(NTT)
            ]
            for t in range(NTT):
                nc.vector.memset(va_sb[t][:, HD : HD + 1], 1.0)
                nc.vector.memset(va_sb[t][:, 2 * HD + 1 : 2 * HD + 2], 1.0)

            hrows = (slice(0, HD), slice(HD, 128))

            # ---------- emit helpers ----------
            def emit_proj_qk(s):
                sg = ext.tile([128, 2 * SW], F32, tag="ex", name="ps_qk")
                for idx, (w_sb, b_sb, dst) in enumerate(
                    ((wq_sb, bq_sb, qt_sb[s]), (wk_sb, bk_sb, kt_sb[s]))
                ):
                    off = idx * SW
                    for c in range(NKC):
                        nc.tensor.matmul(
                            sg[:, off : off + SW],
                            lhsT=w_sb[:, c, :],
                            rhs=xt_sb[c][s][:, :],
                            start=(c == 0),
                            stop=(c == NKC - 1),
                            skip_group_check=True,
                        )
                    nc.vector.tensor_scalar_add(
                        dst[:, :], sg[:, off : off + SW], b_sb[:, :]
                    )

            def emit_proj_v_half(s, half):
                # one full PSUM bank per t-tile: a DVE evac of one t-tile may
                # run while PE still writes another, and PE-write + DVE-read
                # in the SAME bank is a fatal collision
                if True:
                    sg = ext.tile([128, 2 * SW], F32, tag="ex", name="ps_v")
                    for tt in (2 * half, 2 * half + 1):
                        t = 4 * s + tt
                        off = (tt % 2) * SW
                        for c in range(NKC):
                            nc.tensor.matmul(
                                sg[:, off : off + 128],
                                lhsT=xt_sb[c][s][:, tt * 128 : (tt + 1) * 128],
                                rhs=wv_sb[:, c, :],
                                start=(c == 0),
                                stop=(c == NKC - 1),
                                skip_group_check=True,
                            )
                        # evac + b_V add in one op: dst [128,2,64] strided
                        # (skip the ones columns), src/bias [128,2,64]
                        dst3 = va_sb[t][:, 0 : 2 * (HD + 1)].rearrange(
                            "p (a b) -> p a b", a=2
                        )[:, :, 0:HD]
                        src3 = sg[:, off : off + 128].rearrange(
                            "p (a b) -> p a b", a=2
                        )
                        bv3 = bvrep_sb[:, :].rearrange("p (a b) -> p a b", a=2)
                        nc.vector.tensor_add(dst3, src3, bv3)

            def emit_scores(unit):
                s, kb, n, qlo = unit[:4]
                qs = s * SW
                diag = kb * 128 >= qs
                sg = ps.tile([128, 2 * SW], F32, tag="sg", name="ps_sg")
                unit[4] = sg
                for h in range(2):
                    off = h * SW
                    nc.tensor.matmul(
                        sg[:, off : off + n],
                        lhsT=kt_sb[kb // 4][
                            hrows[h], (kb % 4) * 128 : (kb % 4 + 1) * 128
                        ],
                        rhs=qt_sb[s][hrows[h], qlo - qs : qlo - qs + n],
                        start=True,
                        stop=not diag,
                        skip_group_check=True,
                        tile_position=(h * HD, 0),
                    )
                if diag:
                    for h in range(2):
                        nc.tensor.matmul(
                            sg[:, h * SW : h * SW + 128],
                            lhsT=ident_sb[:, :],
                            rhs=mask_sb[:, :],
                            start=False,
                            stop=True,
                            skip_group_check=True,
                        )

            def emit_exp(unit):
                s, kb, n, qlo, sg = unit[:5]
                pt = ptp.tile([128, 2 * SW], BF16, tag="pt", name="pt")
                in3 = sg[:, :].rearrange("p (a b) -> p a b", a=2)[:, :, 0:n]
                out3 = pt[:, 0 : 2 * n].rearrange("p (a b) -> p a b", a=2)
                nc.scalar.activation(
                    out=out3,
                    in_=in3,
                    func=mybir.ActivationFunctionType.Exp,
                    scale=0.125,
                )
                unit[4] = pt

            def emit_pv(unit, zaug):
                s, kb, n, qlo, pt = unit[:5]
                qs = s * SW
                nkb = 4 * (s + 1)
                for h in range(2):
                    nc.tensor.matmul(
                        zaug[h][0 : HD + 1, qlo - qs : SW],
                        lhsT=va_sb[kb][:, h * (HD + 1) : (h + 1) * (HD + 1)],
                        rhs=pt[:, h * n : h * n + n],
                        start=(kb == 0),
                        stop=(kb == nkb - 1),
                        skip_group_check=True,
                    )

            # O-proj work items: (zpair, li_pair_or_ri_pair, qs, j, ready_at).
            # Drained one q-tile per stream iteration once ready_at passes,
            # so the 1.3us of DVE evac per q-tile never bunches up.  The
            # reciprocals are deferred to the first q-tile item (j==0): in
            # emit_zfinish they would block the DVE FIFO on the L-spread
            # DRAM round-trip.
            oproj_work = []

            def emit_zfinish(sp, zaug, g0=0, last=False):
                """Evacuate slice sp's Z/L accumulators and set up the
                deferred, post-O-proj normalisation: unnormalised Z pair
                stacked [128,512] bf16 (head B shifted to partitions
                64..127 via GpSimd) + per-q-tile [128,1] 1/L spreads."""
                zpair = znp.tile([128, SW], BF16, tag="zn")
                znb = slp.tile([HD, SW], BF16, tag="znb")
                nc.vector.tensor_copy(zpair[0:HD, :], zaug[0][0:HD, :])
                nc.vector.tensor_copy(znb[:, :], zaug[1][0:HD, :])
                lri = []
                for h in range(2):
                    lr = slp.tile([1, SW], F32, tag=f"lr{h}", name="lrow")
                    if last:
                        # ScalarE is idle at the tail; take it off DVE
                        nc.scalar.copy(lr[:, :], zaug[h][HD : HD + 1, :])
                    else:
                        nc.vector.tensor_copy(lr[:, :], zaug[h][HD : HD + 1, :])
                    rd = drp.tile([1, SW], F32, tag=f"rd{h}", name="rd")
                    nc.sync.dma_start(out=rd[:, :], in_=lr[:, :])
                    # [1,512] -> [128,4]: partition p, col j = L[j*128+p]
                    li = znp.tile([128, SW // 128], F32, tag=f"li{h}", name="li")
                    nc.sync.dma_start(
                        out=li[:, :],
                        in_=rd[0, :].rearrange("(a p) -> p a", p=128),
                    )
                    lri.append(li)
                nc.gpsimd.dma_start(out=zpair[HD:128, :], in_=znb[:, :])
                state = {"li": lri, "ri": None}
                for j in range(4):
                    oproj_work.append((zpair, state, sp * SW, j, g0 + 10 + 2 * j))

            def emit_oproj_qtile(item, scalar_assist=False, pool=None):
                zpair, state, qs_t, j, _ = item
                if state["ri"] is None:
                    ra = znp.tile([128, SW // 128], F32, tag="ri0", name="ri")
                    nc.vector.reciprocal(ra[:, :], state["li"][0][:, :])
                    rb = znp.tile([128, SW // 128], F32, tag="ri1", name="ri")
                    nc.vector.reciprocal(rb[:, :], state["li"][1][:, :])
                    state["ri"] = (ra, rb)
                ra, rb = state["ri"]
                op = (pool or ext).tile(
                    [128, 2 * SW], F32,
                    tag="ex" if pool is None else "sg",
                    name="ps_o",
                )
                jq = slice(j * 128, (j + 1) * 128)
                nc.tensor.matmul(
                    op[:, 0:SW],
                    lhsT=zpair[0:HD, jq],
                    rhs=wo_sb[0:HD, :],
                    start=True,
                    stop=True,
                    skip_group_check=True,
                    tile_position=(0, 0),
                )
                nc.tensor.matmul(
                    op[:, SW : 2 * SW],
                    lhsT=zpair[HD:128, jq],
                    rhs=wo_sb[HD:128, :],
                    start=True,
                    stop=True,
                    skip_group_check=True,
                    tile_position=(HD, 0),
                )
                o_tmp = outp.tile([128, D], F32, tag="otmp", name="o_tmp")
                if scalar_assist:
                    nc.scalar.mul(o_tmp[:, :], op[:, SW : 2 * SW], rb[:, j : j + 1])
                else:
                    nc.vector.tensor_scalar_mul(
                        o_tmp[:, :], op[:, SW : 2 * SW], rb[:, j : j + 1]
                    )
                o_sb = outp.tile([128, D], BF16, tag="ot", name="o_sb")
                nc.vector.scalar_tensor_tensor(
                    o_sb[:, :],
                    op[:, 0:SW],
                    ra[:, j : j + 1],
                    o_tmp[:, :],
                    op0=mybir.AluOpType.mult,
                    op1=mybir.AluOpType.add,
                )
                r0 = qs_t + j * 128
                nc.sync.dma_start(out=out[r0 : r0 + 128, :], in_=o_sb[:, :])

            # HAM warm-up: the PE clock gate needs ~3.4us of sustained
            # activity to reach 2.4GHz; burn the DMA-wait on dummy matmuls
            # (dep only on the tiny ident/mask DMAs, which are issued first)
            # so slice 0 runs warm and the real work is never behind them.
            warm_ps = ext.tile([128, 2 * SW], F32, tag="ex", name="ps_warm")
            for _ in range(32):
                nc.tensor.matmul(
                    warm_ps[:, 0:128],
                    lhsT=ident_sb[:, :],
                    rhs=mask_sb[:, :],
                    start=True,
                    stop=True,
                    skip_group_check=True,
                )

            # ---------- global unit stream ----------
            stream = []
            first_of_slice = {}
            for s in range(NS):
                qs = s * SW
                first_of_slice[s] = len(stream)
                for kb in range(4 * (s + 1)):
                    qlo = max(qs, kb * 128)
                    stream.append([s, kb, qs + SW - qlo, qlo, None])
            G = len(stream)

            def inserts(s, i, g):
                # extras schedule (one PSUM "ex" slot each, at most ~1/iter):
                # proj for slice s+1 must be fully emitted before the scores
                # of s+1 u0 (iter L-1); V halves feed PV with plenty of lag
                if s == 0:
                    if i == 0:
                        emit_proj_v_half(0, 0)
                    elif i == 1:
                        emit_proj_v_half(0, 1)
                    elif i == 2:
                        emit_proj_qk(1)
                    elif i == 3:
                        emit_proj_v_half(1, 0)
                elif s == 1:
                    if i == 0:
                        emit_proj_v_half(1, 1)
                    elif i == 5:
                        emit_proj_qk(2)
                    elif i == 6:
                        emit_proj_v_half(2, 0)
                    elif i == 7:
                        emit_proj_v_half(2, 1)
                elif s + 1 < NS:
                    if i == 5:
                        emit_proj_qk(s + 1)
                    elif i == 6:
                        emit_proj_v_half(s + 1, 0)
                    elif i == 7:
                        emit_proj_v_half(s + 1, 1)
                if oproj_work and g >= oproj_work[0][4]:
                    emit_oproj_qtile(oproj_work.pop(0))

            emit_proj_qk(0)
            emit_scores(stream[0])
            zaug = None
            prev_zaug = None
            for g in range(G):
                s, kb = stream[g][0], stream[g][1]
                emit_exp(stream[g])
                if g + 1 < G:
                    emit_scores(stream[g + 1])
                if g >= 1:
                    ps_, pkb = stream[g - 1][0], stream[g - 1][1]
                    if pkb == 0:
                        # stream[g-1] opens slice ps_: finish the previous
                        # slice's accumulators, then claim fresh ones
                        if prev_zaug is not None:
                            emit_zfinish(ps_ - 1, prev_zaug, g0=g)
                        zaug = [
                            zps.tile([HD + 1, SW], F32, tag="za", name="zauga"),
                            zps.tile([HD + 1, SW], F32, tag="zb", name="zaugb"),
                        ]
                        prev_zaug = zaug
                    emit_pv(stream[g - 1], zaug)
                inserts(s, g - first_of_slice[s], g)
            emit_pv(stream[G - 1], zaug)

            # tail: last slice's Z finish + remaining O-projections
            emit_zfinish(NS - 1, zaug, g0=G, last=True)
            while oproj_work:
                emit_oproj_qtile(
                    oproj_work.pop(0), scalar_assist=True, pool=ps
                )

    _split_waits(nc)
    return nc


_NC_CACHE = {}


def _get_nc():
    if "nc" not in _NC_CACHE:
        _NC_CACHE["nc"] = build_nc()
    return _NC_CACHE["nc"]


def make_in_maps(combined_embed, W_K, b_K, W_Q, b_Q, W_V, b_V, W_O, b_O):
    f32 = np.float32
    in_maps = []
    for c in range(8):
        b = c // 4
        g = c % 4
        sl = slice(g * 128, (g + 1) * 128)
        xt = np.ascontiguousarray(np.asarray(combined_embed[b], f32).T)
        in_maps.append(
            {
                "xt": xt.astype(_BF16),
                "wq": np.ascontiguousarray(np.asarray(W_Q, f32)[:, sl]).astype(_BF16),
                "wk": np.ascontiguousarray(np.asarray(W_K, f32)[:, sl]).astype(_BF16),
                "wv": np.ascontiguousarray(np.asarray(W_V, f32)[:, sl]).astype(_BF16),
                "wo": np.ascontiguousarray(np.asarray(W_O, f32)[sl, :]).astype(_BF16),
                "bq": np.asarray(b_Q, f32)[sl].reshape(128, 1).copy(),
                "bk": np.asarray(b_K, f32)[sl].reshape(128, 1).copy(),
                "bv": np.asarray(b_V, f32)[sl].reshape(1, 128).copy(),
            }
        )
    return in_maps


def run_cores(in_maps, **kwargs):
    nc = _get_nc()
    return run_bass_kernel_spmd(nc, in_maps, core_ids=list(range(8)), **kwargs)


def kernel(
    combined_embed, W_K, b_K, W_Q, b_Q, W_V, b_V, W_O, b_O
):  # full inputs -> full output
    in_maps = make_in_maps(
        combined_embed, W_K, b_K, W_Q, b_Q, W_V, b_V, W_O, b_O
    )
    res = run_cores(in_maps)
    out = np.zeros((B, T, D), np.float32)
    for c in range(8):
        out[c // 4] += np.asarray(res.results[c]["out"], np.float32)
    out += np.asarray(b_O, np.float32)[None, None, :]
    return out
